# revision 16
# baseline (speedup 1.0000x reference)
"""Trainium2 Bass kernel for NeuroplasticLlama block-sparse adapter (moe_routing).

Contract: kernel(**inputs) takes FULL unsharded inputs (as produced by
setup_inputs) and returns the FULL [4, 4096, 4096] float32 output.

Strategy (data/sequence parallel over 8 cores, 2048 tokens each):
  - Each core's 2048 contiguous tokens belong to exactly one batch, so the
    task embedding contributes only per-core constant bias vectors
    (te @ A, te @ W2) -- h = x + te is never materialized.
  - Dense formulation of the routed computation:
      scores s[t,n] = x @ (Wp @ centers.T)[:,n] + const_n (shift dropped;
        top-k set and softmax are shift invariant)
      top-3 via threshold = 3rd max (3 rounds of max + mask-out)
      gates g[t,n] = exp(s - max) * (s >= thr3) / sum(...)
      z[t,:] = x @ A_all (all 512 block-rank pairs, dense)
      zg = z * expand4(g);  delta = block-diag(Bm) matmul
  - The device computes only DELTA (fp8 in, fp8 out); the residual
    y = x + delta is applied on the host during unsharding, keeping the
    x term exact and HBM traffic at 2 MB in + 2 MB out per 512-token
    macrotile per core.
  - x is fed pre-transposed and pre-shuffled to the [p][mt][k][t] layout
    so every DMA is a contiguous [128, N] transfer (128 descriptors; a
    strided [H, tokens] slice costs ~18us of descriptor generation).
  - scores and z are fp8 DoubleRow matmuls off the same x tile; delta is
    bf16 (output-stream bound, DoubleRow would not help).
  - SOFTWARE PIPELINE: macrotile m's delta/gate-apply phase executes
    during macrotile m+1's scores/z phase, with delta pair-matmuls
    interleaved between z chunks.  The PSUM->SBUF delta drains (the
    throughput-critical ~9us/mt of scalar+vector work) then overlap the
    z matmuls instead of bursting at the end of each macrotile, and the
    gating chain of m+1 runs in the vector queue after m's drains.
"""

import sys

if "/opt/trn_rl_repo" not in sys.path:
    sys.path.insert(0, "/opt/trn_rl_repo")

import numpy as np
import ml_dtypes

H = 4096
NB = 128
BLK = 32
R = 4
B = 4
S = 4096
NCORES = 8
TPC = (B * S) // NCORES  # tokens per core = 2048
T = 512                  # tokens per macrotile
NMT = TPC // T           # 4 macrotiles per core
NKT = H // 128           # 32 k-tiles over the hidden dim
NP = NKT // 2            # 16 DoubleRow k-pair tiles
BIG = 1.0e30

TRACE = False            # set by test.py for profiling runs
TRACE_DIR = None
LAST_RESULT = None       # BassKernelResults of the last run

_COMPILED = None


def _build():
    import concourse.bacc as bacc
    import concourse.tile as tile
    from concourse import mybir, masks

    f32 = mybir.dt.float32
    bf16 = mybir.dt.bfloat16
    f8 = mybir.dt.float8e4
    AF = mybir.ActivationFunctionType
    AL = mybir.AluOpType
    AX = mybir.AxisListType
    DR = mybir.MatmulPerfMode.DoubleRow

    nc = bacc.Bacc("TRN2", target_bir_lowering=False, debug=False,
                   num_devices=NCORES)

    xt_d = nc.dram_tensor("xt", [128, NMT * NKT * T], f8, kind="ExternalInput")
    ah_d = nc.dram_tensor("ah", [128, 4 * NKT * 128], f8, kind="ExternalInput")
    ws_d = nc.dram_tensor("ws", [128, NKT * 128], f8, kind="ExternalInput")
    bpk_d = nc.dram_tensor("bpk", [128, NKT * 128], bf16, kind="ExternalInput")
    e_d = nc.dram_tensor("e", [128, 512], bf16, kind="ExternalInput")
    bias_d = nc.dram_tensor("bias", [128, 5], f32, kind="ExternalInput")
    yt_d = nc.dram_tensor("yt", [128, NMT * NKT * T], f8, kind="ExternalOutput")

    xt_ap = xt_d.ap()
    yt_ap = yt_d.ap()

    with tile.TileContext(nc) as tc:
        from contextlib import ExitStack
        with ExitStack() as ctx:
            cpool = ctx.enter_context(tc.tile_pool(name="consts", bufs=1))
            xpool = ctx.enter_context(tc.tile_pool(name="xg", bufs=2))
            dpool = ctx.enter_context(tc.tile_pool(name="dall", bufs=2))
            zpool = ctx.enter_context(tc.tile_pool(name="zb", bufs=8))
            gpool = ctx.enter_context(tc.tile_pool(name="gate", bufs=3))
            spool = ctx.enter_context(tc.tile_pool(name="scal", bufs=4))
            pp = ctx.enter_context(tc.tile_pool(name="ps", bufs=2, space="PSUM"))

            # ---- persistent constants ----
            # xa0 is issued FIRST (in the macrotile loop below) on the sync
            # HWDGE ring; az/bpk/esb queue BEHIND it on the same ring so the
            # first scores matmul's input gets the full HBM bandwidth
            # (per-queue FIFO), while the tiny ws/bias go in parallel on
            # scalar/gpsimd.  az0 is only needed ~4us after scores start.
            ws = cpool.tile([128, NKT * 128], f8, name="ws", tag="ws")
            nc.scalar.dma_start(ws[:], ws_d.ap()[:])
            bias = cpool.tile([128, 5], f32, name="bias", tag="bias")
            nc.gpsimd.dma_start(bias[:], bias_d.ap()[:])
            ident = cpool.tile([128, 128], bf16, name="ident", tag="ident")
            masks.make_identity(nc, ident[:])
            az = []
            bpk = cpool.tile([128, NKT * 128], bf16, name="bpk", tag="bpk")
            esb = cpool.tile([128, 512], bf16, name="esb", tag="esb")

            def emit_const_loads():
                for q in range(4):
                    t_az = cpool.tile([128, NKT * 128], f8, name=f"az{q}",
                                      tag=f"az{q}")
                    nc.sync.dma_start(
                        t_az[:],
                        ah_d.ap()[:, q * NKT * 128:(q + 1) * NKT * 128])
                    az.append(t_az)
                nc.sync.dma_start(bpk[:], bpk_d.ap()[:])
                nc.sync.dma_start(esb[:], e_d.ap()[:])

            NTS = T // 128  # token sub-tiles per macrotile

            # ---------------- pipelined stage helpers ----------------
            def emit_scores(xa):
                sp = pp.tile([128, T], f32, space="PSUM", name="sp", tag="zp")
                for k2 in range(NP):
                    nc.tensor.matmul(
                        sp[:],
                        ws[:, k2 * 256:(k2 + 1) * 256]
                        .rearrange("p (two m) -> p two m", two=2),
                        xa[:, 2 * k2 * T:(2 * k2 + 2) * T]
                        .rearrange("p (two t) -> p two t", two=2),
                        start=(k2 == 0), stop=(k2 == NP - 1),
                        perf_mode=DR,
                    )
                s_sb = gpool.tile([128, T], bf16, name="s_sb", tag="s_sb")
                nc.scalar.activation(s_sb[:], sp[:], AF.Identity,
                                     bias=bias[:, 4:5], scale=1.0)
                return s_sb

            def emit_transpose_scores(s_sb):
                s_ps = pp.tile([128, T], bf16, space="PSUM", name="s_ps",
                               tag="tr", bufs=1)
                for ts in range(NTS):
                    nc.tensor.transpose(s_ps[:, ts * 128:(ts + 1) * 128],
                                        s_sb[:, ts * 128:(ts + 1) * 128],
                                        ident[:])
                stn_all = gpool.tile([128, T], f32, name="stn_all", tag="stn",
                                     bufs=2)
                nc.scalar.copy(stn_all[:], s_ps[:])
                return stn_all

            def emit_transpose_gates(ggs):
                g_ps = pp.tile([128, T], bf16, space="PSUM", name="g_ps",
                               tag="tr", bufs=1)
                for ts in range(NTS):
                    nc.tensor.transpose(g_ps[:, ts * 128:(ts + 1) * 128],
                                        ggs[ts][:], ident[:])
                gt_sb = gpool.tile([128, T], bf16, name="gt_sb", tag="gt_sb")
                nc.scalar.copy(gt_sb[:], g_ps[:])
                return gt_sb

            def emit_z_half(xa, q, half, zp):
                for k2 in range(NP // 2 * half, NP // 2 * (half + 1)):
                    nc.tensor.matmul(
                        zp[:],
                        az[q][:, k2 * 256:(k2 + 1) * 256]
                        .rearrange("p (two m) -> p two m", two=2),
                        xa[:, 2 * k2 * T:(2 * k2 + 2) * T]
                        .rearrange("p (two t) -> p two t", two=2),
                        start=(k2 == 0), stop=(k2 == NP - 1),
                        perf_mode=DR,
                    )

            def emit_gx_mul(st, q):
                gx = pp.tile([128, T], f32, space="PSUM", name="gx", tag="gx",
                             bufs=1)
                nc.tensor.matmul(gx[:],
                                 esb[:, q * 128:(q + 1) * 128],
                                 st["gt"][:],
                                 start=True, stop=True)
                nc.vector.tensor_mul(st["zbs"][q][:], st["zbs"][q][:], gx[:])

            def emit_delta_pairs(st, q, prs, veceng):
                # delta pair matmuls + PSUM->SBUF fp8 drains for pair
                # indices prs of quarter q of the previous macrotile
                for pr in prs:
                    hc = q * 8 + pr * 2
                    dp = pp.tile([128, 2 * T], f32, space="PSUM", name="dp",
                                 tag="dp", bufs=2)
                    for hf in range(2):
                        nc.tensor.matmul(
                            dp[:, hf * T:(hf + 1) * T],
                            bpk[:, (hc + hf) * 128:(hc + hf + 1) * 128],
                            st["zbs"][q][:],
                            start=True, stop=True)
                    if veceng == "both":
                        # tail drain: no later z-phase to hide behind, so
                        # both engines drain one half of every pair
                        nc.scalar.copy(st["da"][:, hc * T:(hc + 1) * T],
                                       dp[:, 0:T])
                        nc.vector.tensor_copy(
                            st["da"][:, (hc + 1) * T:(hc + 2) * T],
                            dp[:, T:2 * T])
                    elif pr in veceng:
                        nc.vector.tensor_copy(st["da"][:, hc * T:(hc + 2) * T],
                                              dp[:])
                    else:
                        nc.scalar.copy(st["da"][:, hc * T:(hc + 2) * T],
                                       dp[:])

            def emit_store(st, q):
                smt = st["mt"]
                nc.sync.dma_start(
                    yt_ap[:, (smt * 4 + q) * 8 * T:(smt * 4 + q + 1) * 8 * T],
                    st["da"][:, q * 8 * T:(q + 1) * 8 * T],
                )

            def emit_chain_ts(stn_all, ts):
                # the DVE gating chain for one 128-token sub-tile
                if True:
                    stn = stn_all[:, ts * 128:(ts + 1) * 128]
                    r1 = spool.tile([128, 1], f32, name="r1", tag="r1")
                    nc.vector.reduce_max(r1[:], stn, axis=AX.X)
                    mb1 = gpool.tile([128, 128], f32, name="mb1", tag="mb1")
                    nc.vector.tensor_scalar(mb1[:], stn, r1[:], BIG,
                                            AL.is_ge, AL.mult)
                    s2 = gpool.tile([128, 128], f32, name="s2", tag="s2")
                    nc.vector.tensor_sub(s2[:], stn, mb1[:])
                    r2 = spool.tile([128, 1], f32, name="r2", tag="r2")
                    nc.vector.reduce_max(r2[:], s2[:], axis=AX.X)
                    mb2 = gpool.tile([128, 128], f32, name="mb2", tag="mb2")
                    nc.vector.tensor_scalar(mb2[:], s2[:], r2[:], BIG,
                                            AL.is_ge, AL.mult)
                    s3 = gpool.tile([128, 128], f32, name="s3", tag="s3")
                    nc.vector.tensor_sub(s3[:], s2[:], mb2[:])
                    r3 = spool.tile([128, 1], f32, name="r3", tag="r3")
                    nc.vector.reduce_max(r3[:], s3[:], axis=AX.X)
                    nr1 = spool.tile([128, 1], f32, name="nr1", tag="nr1")
                    nc.vector.tensor_scalar_mul(nr1[:], r1[:], -1.0)
                    ex = gpool.tile([128, 128], f32, name="ex", tag="ex")
                    nc.scalar.activation(ex[:], stn, AF.Exp, bias=nr1[:],
                                         scale=1.0)
                    em = gpool.tile([128, 128], f32, name="em", tag="em")
                    zs = spool.tile([128, 1], f32, name="zs", tag="zs")
                    nc.vector.scalar_tensor_tensor(em[:], stn, r3[:], ex[:],
                                                   AL.is_ge, AL.mult,
                                                   accum_out=zs[:])
                    rz = spool.tile([128, 1], f32, name="rz", tag="rz")
                    nc.vector.reciprocal(rz[:], zs[:])
                    gg = gpool.tile([128, 128], bf16, name="gg", tag="gg",
                                    bufs=NTS + 1)
                    nc.vector.tensor_scalar_mul(gg[:], em[:], rz[:])
                    return gg

            # ---------------- the pipelined macrotile loop ----------------
            prev = None  # state of the previous macrotile
            for mt in range(NMT):
                xa = xpool.tile([128, NKT * T], f8, name="xa", tag="xa")
                nc.sync.dma_start(
                    xa[:], xt_ap[:, mt * NKT * T:(mt + 1) * NKT * T])
                if mt == 0:
                    emit_const_loads()

                s_sb = emit_scores(xa)
                stn_all = emit_transpose_scores(s_sb)
                if prev is not None:
                    prev["gt"] = emit_transpose_gates(prev["ggs"])

                da = dpool.tile([128, NKT * T], f8, name="da", tag="da")
                cur = {"mt": mt, "da": da, "zbs": [], "ggs": None}

                ggs = []
                for q in range(4):
                    if prev is not None:
                        emit_gx_mul(prev, q)
                        emit_delta_pairs(prev, q, (0, 1), veceng={1})
                    zp = pp.tile([128, T], f32, space="PSUM", name="zp",
                                 tag="zp")
                    emit_z_half(xa, q, 0, zp)
                    if prev is not None:
                        emit_delta_pairs(prev, q, (2, 3), veceng={3})
                    emit_z_half(xa, q, 1, zp)
                    if prev is not None:
                        emit_store(prev, q)
                    zb = zpool.tile([128, T], bf16, name="zb", tag="zb")
                    nc.scalar.activation(zb[:], zp[:], AF.Identity,
                                         bias=bias[:, q:q + 1], scale=1.0)
                    cur["zbs"].append(zb)
                    # gating sub-chain for sub-tile q goes into the vector
                    # queue here so the chain finishes alongside the z phase
                    ggs.append(emit_chain_ts(stn_all, q))

                cur["ggs"] = ggs
                prev = cur

            # ---- drain the pipeline: delta of the last macrotile ----
            prev["gt"] = emit_transpose_gates(prev["ggs"])
            for q in range(4):
                emit_gx_mul(prev, q)
                emit_delta_pairs(prev, q, (0, 1, 2, 3), veceng="both")
                emit_store(prev, q)

    nc.compile()
    return nc


def _prep_consts(task_emb, task_ids, Wp, bp, centers, A, Bm, adapter_scale):
    scale = float(np.asarray(adapter_scale))
    A_all = np.ascontiguousarray(
        A.transpose(1, 0, 2).reshape(H, NB * R).astype(np.float32))
    W2 = (Wp @ centers.T).astype(np.float32)                     # [H, 128]

    # ah: [p, q, k2, two, m] = A_all[(2*k2+two)*128+p, q*128+m], fp8 e4m3
    # (DoubleRow pairs of consecutive k-tiles interleave along the free dim)
    ah = (A_all.reshape(NKT, 128, 4, 128).transpose(1, 2, 0, 3)
          .reshape(128, 4 * NKT * 128).astype(ml_dtypes.float8_e4m3))
    ah = np.ascontiguousarray(ah)
    # ws: [p, k2, two, m] = W2[(2*k2+two)*128+p, m], fp8 (DoubleRow pairs)
    wsn = np.ascontiguousarray(
        W2.reshape(NKT, 128, 128).transpose(1, 0, 2).reshape(128, NKT * 128)
        .astype(ml_dtypes.float8_e4m3))

    # block-diag up-projection, K=128 per h-chunk
    bpk = np.zeros((128, NKT * 128), np.float32)
    for hc in range(NKT):
        for mblk in range(4):
            n = hc * 4 + mblk
            for r in range(R):
                row = (hc % 8) * 16 + mblk * 4 + r
                bpk[row, hc * 128 + mblk * 32: hc * 128 + mblk * 32 + 32] = \
                    Bm[n, r, :] * scale
    bpk = bpk.astype(ml_dtypes.bfloat16)

    e_np = (np.arange(128)[:, None] == (np.arange(512)[None, :] // 4)) \
        .astype(ml_dtypes.bfloat16)

    sconst = (bp @ centers.T - 0.5 * (centers ** 2).sum(-1)).astype(np.float32)

    biases = []
    for c in range(NCORES):
        te = task_emb[int(np.asarray(task_ids)[c // 2])].astype(np.float32)
        b5 = np.empty((128, 5), np.float32)
        zoff = te @ A_all                                        # [512]
        for q in range(4):
            b5[:, q] = zoff[q * 128:(q + 1) * 128]
        b5[:, 4] = te @ W2 + sconst
        biases.append(np.ascontiguousarray(b5))
    return ah, wsn, bpk, e_np, biases


def kernel(x, task_ids, task_emb, Wp, bp, centers, A, Bm, adapter_scale):
    global _COMPILED, LAST_RESULT
    from concourse import bass_utils

    x = np.asarray(x, dtype=np.float32)
    task_ids = np.asarray(task_ids)
    task_emb = np.asarray(task_emb, dtype=np.float32)
    Wp = np.asarray(Wp, dtype=np.float32)
    bp = np.asarray(bp, dtype=np.float32)
    centers = np.asarray(centers, dtype=np.float32)
    A = np.asarray(A, dtype=np.float32)
    Bm = np.asarray(Bm, dtype=np.float32)

    if _COMPILED is None:
        _COMPILED = _build()
    nc = _COMPILED

    ah, wsn, bpk, e_np, biases = _prep_consts(
        task_emb, task_ids, Wp, bp, centers, A, Bm, adapter_scale)

    xf = x.reshape(B * S, H)
    xf8 = xf.astype(ml_dtypes.float8_e4m3)
    in_maps = []
    for c in range(NCORES):
        xtc = xf8[c * TPC:(c + 1) * TPC].reshape(NMT, T, NKT, 128)
        xtc = np.ascontiguousarray(xtc.transpose(3, 0, 2, 1)) \
            .reshape(128, NMT * NKT * T)
        in_maps.append({"xt": xtc, "ah": ah, "ws": wsn, "bpk": bpk,
                       "e": e_np, "bias": biases[c]})

    kwargs = {}
    if TRACE:
        kwargs = dict(trace=True, tmpdir=TRACE_DIR)
    res = bass_utils.run_bass_kernel_spmd(
        nc, in_maps, core_ids=list(range(NCORES)), **kwargs)
    LAST_RESULT = res

    out = np.empty((B * S, H), np.float32)
    for c in range(NCORES):
        dat = res.results[c]["yt"].reshape(128, NMT, 4, 8, T)
        delta = dat.transpose(1, 4, 2, 3, 0).reshape(TPC, H)
        out[c * TPC:(c + 1) * TPC] = xf[c * TPC:(c + 1) * TPC] + \
            delta.astype(np.float32)
    return out.reshape(B, S, H)


# revision 17
# speedup vs baseline: 1.0217x; 1.0217x over previous
"""Trainium2 Bass kernel for NeuroplasticLlama block-sparse adapter (moe_routing).

Contract: kernel(**inputs) takes FULL unsharded inputs (as produced by
setup_inputs) and returns the FULL [4, 4096, 4096] float32 output.

Strategy (data/sequence parallel over 8 cores, 2048 tokens each):
  - Each core's 2048 contiguous tokens belong to exactly one batch, so the
    task embedding contributes only per-core constant bias vectors
    (te @ A, te @ W2) -- h = x + te is never materialized.
  - Dense formulation of the routed computation:
      scores s[t,n] = x @ (Wp @ centers.T)[:,n] + const_n (shift dropped;
        top-k set and softmax are shift invariant)
      top-3 via threshold = 3rd max (3 rounds of max + mask-out)
      gates g[t,n] = exp(s - max) * (s >= thr3) / sum(...)
      z[t,:] = x @ A_all (all 512 block-rank pairs, dense)
      zg = z * expand4(g);  delta = block-diag(Bm) matmul
  - The device computes only DELTA (fp8 in, fp8 out); the residual
    y = x + delta is applied on the host during unsharding, keeping the
    x term exact and HBM traffic at 2 MB in + 2 MB out per 512-token
    macrotile per core.
  - x is fed pre-transposed and pre-shuffled to the [p][mt][k][t] layout
    so every DMA is a contiguous [128, N] transfer (128 descriptors; a
    strided [H, tokens] slice costs ~18us of descriptor generation).
  - scores and z are fp8 DoubleRow matmuls off the same x tile; delta is
    bf16 (output-stream bound, DoubleRow would not help).
  - SOFTWARE PIPELINE: macrotile m's delta/gate-apply phase executes
    during macrotile m+1's scores/z phase, with delta pair-matmuls
    interleaved between z chunks.  The PSUM->SBUF delta drains (the
    throughput-critical ~9us/mt of scalar+vector work) then overlap the
    z matmuls instead of bursting at the end of each macrotile, and the
    gating chain of m+1 runs in the vector queue after m's drains.
"""

import sys

if "/opt/trn_rl_repo" not in sys.path:
    sys.path.insert(0, "/opt/trn_rl_repo")

import numpy as np
import ml_dtypes

H = 4096
NB = 128
BLK = 32
R = 4
B = 4
S = 4096
NCORES = 8
TPC = (B * S) // NCORES  # tokens per core = 2048
T = 512                  # tokens per macrotile
NMT = TPC // T           # 4 macrotiles per core
NKT = H // 128           # 32 k-tiles over the hidden dim
NP = NKT // 2            # 16 DoubleRow k-pair tiles
BIG = 1.0e30

TRACE = False            # set by test.py for profiling runs
TRACE_DIR = None
LAST_RESULT = None       # BassKernelResults of the last run

_COMPILED = None


def _build():
    import concourse.bacc as bacc
    import concourse.tile as tile
    from concourse import mybir, masks

    f32 = mybir.dt.float32
    bf16 = mybir.dt.bfloat16
    f8 = mybir.dt.float8e4
    AF = mybir.ActivationFunctionType
    AL = mybir.AluOpType
    AX = mybir.AxisListType
    DR = mybir.MatmulPerfMode.DoubleRow

    nc = bacc.Bacc("TRN2", target_bir_lowering=False, debug=False,
                   num_devices=NCORES)

    xt_d = nc.dram_tensor("xt", [128, NMT * NKT * T], f8, kind="ExternalInput")
    ah_d = nc.dram_tensor("ah", [128, 4 * NKT * 128], f8, kind="ExternalInput")
    ws_d = nc.dram_tensor("ws", [128, NKT * 128], f8, kind="ExternalInput")
    bpk_d = nc.dram_tensor("bpk", [128, NKT * 128], bf16, kind="ExternalInput")
    e_d = nc.dram_tensor("e", [128, 512], bf16, kind="ExternalInput")
    bias_d = nc.dram_tensor("bias", [128, 5], f32, kind="ExternalInput")
    yt_d = nc.dram_tensor("yt", [128, NMT * NKT * T], f8, kind="ExternalOutput")

    xt_ap = xt_d.ap()
    yt_ap = yt_d.ap()

    with tile.TileContext(nc) as tc:
        from contextlib import ExitStack
        with ExitStack() as ctx:
            cpool = ctx.enter_context(tc.tile_pool(name="consts", bufs=1))
            xpool = ctx.enter_context(tc.tile_pool(name="xg", bufs=2))
            dpool = ctx.enter_context(tc.tile_pool(name="dall", bufs=2))
            zpool = ctx.enter_context(tc.tile_pool(name="zb", bufs=8))
            gpool = ctx.enter_context(tc.tile_pool(name="gate", bufs=3))
            spool = ctx.enter_context(tc.tile_pool(name="scal", bufs=4))
            pp = ctx.enter_context(tc.tile_pool(name="ps", bufs=2, space="PSUM"))

            # ---- persistent constants ----
            # xa0 is issued FIRST (in the macrotile loop below) on the sync
            # HWDGE ring; az/bpk/esb queue BEHIND it on the same ring so the
            # first scores matmul's input gets the full HBM bandwidth
            # (per-queue FIFO), while the tiny ws/bias go in parallel on
            # scalar/gpsimd.  az0 is only needed ~4us after scores start.
            ws = cpool.tile([128, NKT * 128], f8, name="ws", tag="ws")
            nc.scalar.dma_start(ws[:], ws_d.ap()[:])
            bias = cpool.tile([128, 5], f32, name="bias", tag="bias")
            nc.gpsimd.dma_start(bias[:], bias_d.ap()[:])
            ident = cpool.tile([128, 128], bf16, name="ident", tag="ident")
            masks.make_identity(nc, ident[:])
            az = []
            bpk = cpool.tile([128, NKT * 128], bf16, name="bpk", tag="bpk")
            esb = cpool.tile([128, 512], bf16, name="esb", tag="esb")

            def emit_const_loads():
                for q in range(4):
                    t_az = cpool.tile([128, NKT * 128], f8, name=f"az{q}",
                                      tag=f"az{q}")
                    nc.sync.dma_start(
                        t_az[:],
                        ah_d.ap()[:, q * NKT * 128:(q + 1) * NKT * 128])
                    az.append(t_az)
                nc.sync.dma_start(bpk[:], bpk_d.ap()[:])
                nc.sync.dma_start(esb[:], e_d.ap()[:])

            NTS = T // 128  # token sub-tiles per macrotile

            # ---------------- pipelined stage helpers ----------------
            def emit_scores(xa):
                sp = pp.tile([128, T], f32, space="PSUM", name="sp", tag="zp")
                for k2 in range(NP):
                    nc.tensor.matmul(
                        sp[:],
                        ws[:, k2 * 256:(k2 + 1) * 256]
                        .rearrange("p (two m) -> p two m", two=2),
                        xa[:, 2 * k2 * T:(2 * k2 + 2) * T]
                        .rearrange("p (two t) -> p two t", two=2),
                        start=(k2 == 0), stop=(k2 == NP - 1),
                        perf_mode=DR,
                    )
                s_sb = gpool.tile([128, T], bf16, name="s_sb", tag="s_sb")
                nc.scalar.activation(s_sb[:], sp[:], AF.Identity,
                                     bias=bias[:, 4:5], scale=1.0)
                return s_sb

            def emit_transpose_scores(s_sb):
                s_ps = pp.tile([128, T], bf16, space="PSUM", name="s_ps",
                               tag="tr", bufs=1)
                for ts in range(NTS):
                    nc.tensor.transpose(s_ps[:, ts * 128:(ts + 1) * 128],
                                        s_sb[:, ts * 128:(ts + 1) * 128],
                                        ident[:])
                stn_all = gpool.tile([128, T], f32, name="stn_all", tag="stn",
                                     bufs=2)
                nc.scalar.copy(stn_all[:], s_ps[:])
                return stn_all

            def emit_transpose_gates(ggs):
                g_ps = pp.tile([128, T], bf16, space="PSUM", name="g_ps",
                               tag="tr", bufs=1)
                for ts in range(NTS):
                    nc.tensor.transpose(g_ps[:, ts * 128:(ts + 1) * 128],
                                        ggs[ts][:], ident[:])
                gt_sb = gpool.tile([128, T], bf16, name="gt_sb", tag="gt_sb")
                nc.scalar.copy(gt_sb[:], g_ps[:])
                return gt_sb

            def emit_z_half(xa, q, half, zp):
                for k2 in range(NP // 2 * half, NP // 2 * (half + 1)):
                    nc.tensor.matmul(
                        zp[:],
                        az[q][:, k2 * 256:(k2 + 1) * 256]
                        .rearrange("p (two m) -> p two m", two=2),
                        xa[:, 2 * k2 * T:(2 * k2 + 2) * T]
                        .rearrange("p (two t) -> p two t", two=2),
                        start=(k2 == 0), stop=(k2 == NP - 1),
                        perf_mode=DR,
                    )

            def emit_gx_mul(st, q):
                gx = pp.tile([128, T], f32, space="PSUM", name="gx", tag="gx",
                             bufs=1)
                nc.tensor.matmul(gx[:],
                                 esb[:, q * 128:(q + 1) * 128],
                                 st["gt"][:],
                                 start=True, stop=True)
                nc.vector.tensor_mul(st["zbs"][q][:], st["zbs"][q][:], gx[:])

            def emit_delta_pairs(st, q, prs, veceng):
                # delta pair matmuls + PSUM->SBUF fp8 drains for pair
                # indices prs of quarter q of the previous macrotile
                for pr in prs:
                    hc = q * 8 + pr * 2
                    dp = pp.tile([128, 2 * T], f32, space="PSUM", name="dp",
                                 tag="dp", bufs=2)
                    for hf in range(2):
                        nc.tensor.matmul(
                            dp[:, hf * T:(hf + 1) * T],
                            bpk[:, (hc + hf) * 128:(hc + hf + 1) * 128],
                            st["zbs"][q][:],
                            start=True, stop=True)
                    if veceng == "both":
                        # tail drain: no later z-phase to hide behind, so
                        # both engines drain one half of every pair
                        nc.scalar.copy(st["da"][:, hc * T:(hc + 1) * T],
                                       dp[:, 0:T])
                        nc.vector.tensor_copy(
                            st["da"][:, (hc + 1) * T:(hc + 2) * T],
                            dp[:, T:2 * T])
                    elif pr in veceng:
                        nc.vector.tensor_copy(st["da"][:, hc * T:(hc + 2) * T],
                                              dp[:])
                    else:
                        nc.scalar.copy(st["da"][:, hc * T:(hc + 2) * T],
                                       dp[:])

            def emit_store(st, q):
                smt = st["mt"]
                nc.sync.dma_start(
                    yt_ap[:, (smt * 4 + q) * 8 * T:(smt * 4 + q + 1) * 8 * T],
                    st["da"][:, q * 8 * T:(q + 1) * 8 * T],
                )

            def emit_chain(stn_all):
                # the DVE gating chain for all 4 token sub-tiles
                ggs = []
                for ts in range(NTS):
                    stn = stn_all[:, ts * 128:(ts + 1) * 128]
                    r1 = spool.tile([128, 1], f32, name="r1", tag="r1")
                    nc.vector.reduce_max(r1[:], stn, axis=AX.X)
                    mb1 = gpool.tile([128, 128], f32, name="mb1", tag="mb1")
                    nc.vector.tensor_scalar(mb1[:], stn, r1[:], BIG,
                                            AL.is_ge, AL.mult)
                    s2 = gpool.tile([128, 128], f32, name="s2", tag="s2")
                    nc.vector.tensor_sub(s2[:], stn, mb1[:])
                    r2 = spool.tile([128, 1], f32, name="r2", tag="r2")
                    nc.vector.reduce_max(r2[:], s2[:], axis=AX.X)
                    mb2 = gpool.tile([128, 128], f32, name="mb2", tag="mb2")
                    nc.vector.tensor_scalar(mb2[:], s2[:], r2[:], BIG,
                                            AL.is_ge, AL.mult)
                    s3 = gpool.tile([128, 128], f32, name="s3", tag="s3")
                    nc.vector.tensor_sub(s3[:], s2[:], mb2[:])
                    r3 = spool.tile([128, 1], f32, name="r3", tag="r3")
                    nc.vector.reduce_max(r3[:], s3[:], axis=AX.X)
                    nr1 = spool.tile([128, 1], f32, name="nr1", tag="nr1")
                    nc.vector.tensor_scalar_mul(nr1[:], r1[:], -1.0)
                    ex = gpool.tile([128, 128], f32, name="ex", tag="ex")
                    nc.scalar.activation(ex[:], stn, AF.Exp, bias=nr1[:],
                                         scale=1.0)
                    em = gpool.tile([128, 128], f32, name="em", tag="em")
                    zs = spool.tile([128, 1], f32, name="zs", tag="zs")
                    nc.vector.scalar_tensor_tensor(em[:], stn, r3[:], ex[:],
                                                   AL.is_ge, AL.mult,
                                                   accum_out=zs[:])
                    rz = spool.tile([128, 1], f32, name="rz", tag="rz")
                    nc.vector.reciprocal(rz[:], zs[:])
                    gg = gpool.tile([128, 128], bf16, name="gg", tag="gg",
                                    bufs=NTS + 1)
                    nc.vector.tensor_scalar_mul(gg[:], em[:], rz[:])
                    ggs.append(gg)
                return ggs

            # ---------------- the pipelined macrotile loop ----------------
            prev = None  # state of the previous macrotile
            for mt in range(NMT):
                xa = xpool.tile([128, NKT * T], f8, name="xa", tag="xa")
                nc.sync.dma_start(
                    xa[:], xt_ap[:, mt * NKT * T:(mt + 1) * NKT * T])
                if mt == 0:
                    emit_const_loads()

                s_sb = emit_scores(xa)
                stn_all = emit_transpose_scores(s_sb)
                if prev is not None:
                    prev["gt"] = emit_transpose_gates(prev["ggs"])

                da = dpool.tile([128, NKT * T], f8, name="da", tag="da")
                cur = {"mt": mt, "da": da, "zbs": [], "ggs": None}

                for q in range(4):
                    if prev is not None:
                        emit_gx_mul(prev, q)
                        emit_delta_pairs(prev, q, (0, 1), veceng={1})
                    zp = pp.tile([128, T], f32, space="PSUM", name="zp",
                                 tag="zp")
                    emit_z_half(xa, q, 0, zp)
                    if prev is not None:
                        emit_delta_pairs(prev, q, (2, 3), veceng={3})
                    emit_z_half(xa, q, 1, zp)
                    if prev is not None:
                        emit_store(prev, q)
                    zb = zpool.tile([128, T], bf16, name="zb", tag="zb")
                    nc.scalar.activation(zb[:], zp[:], AF.Identity,
                                         bias=bias[:, q:q + 1], scale=1.0)
                    cur["zbs"].append(zb)

                cur["ggs"] = emit_chain(stn_all)
                prev = cur

            # ---- drain the pipeline: delta of the last macrotile ----
            prev["gt"] = emit_transpose_gates(prev["ggs"])
            for q in range(4):
                emit_gx_mul(prev, q)
                emit_delta_pairs(prev, q, (0, 1, 2, 3), veceng="both")
                emit_store(prev, q)

    nc.compile()
    return nc


def _prep_consts(task_emb, task_ids, Wp, bp, centers, A, Bm, adapter_scale):
    scale = float(np.asarray(adapter_scale))
    A_all = np.ascontiguousarray(
        A.transpose(1, 0, 2).reshape(H, NB * R).astype(np.float32))
    W2 = (Wp @ centers.T).astype(np.float32)                     # [H, 128]

    # ah: [p, q, k2, two, m] = A_all[(2*k2+two)*128+p, q*128+m], fp8 e4m3
    # (DoubleRow pairs of consecutive k-tiles interleave along the free dim)
    ah = (A_all.reshape(NKT, 128, 4, 128).transpose(1, 2, 0, 3)
          .reshape(128, 4 * NKT * 128).astype(ml_dtypes.float8_e4m3))
    ah = np.ascontiguousarray(ah)
    # ws: [p, k2, two, m] = W2[(2*k2+two)*128+p, m], fp8 (DoubleRow pairs)
    wsn = np.ascontiguousarray(
        W2.reshape(NKT, 128, 128).transpose(1, 0, 2).reshape(128, NKT * 128)
        .astype(ml_dtypes.float8_e4m3))

    # block-diag up-projection, K=128 per h-chunk
    bpk = np.zeros((128, NKT * 128), np.float32)
    for hc in range(NKT):
        for mblk in range(4):
            n = hc * 4 + mblk
            for r in range(R):
                row = (hc % 8) * 16 + mblk * 4 + r
                bpk[row, hc * 128 + mblk * 32: hc * 128 + mblk * 32 + 32] = \
                    Bm[n, r, :] * scale
    bpk = bpk.astype(ml_dtypes.bfloat16)

    e_np = (np.arange(128)[:, None] == (np.arange(512)[None, :] // 4)) \
        .astype(ml_dtypes.bfloat16)

    sconst = (bp @ centers.T - 0.5 * (centers ** 2).sum(-1)).astype(np.float32)

    biases = []
    for c in range(NCORES):
        te = task_emb[int(np.asarray(task_ids)[c // 2])].astype(np.float32)
        b5 = np.empty((128, 5), np.float32)
        zoff = te @ A_all                                        # [512]
        for q in range(4):
            b5[:, q] = zoff[q * 128:(q + 1) * 128]
        b5[:, 4] = te @ W2 + sconst
        biases.append(np.ascontiguousarray(b5))
    return ah, wsn, bpk, e_np, biases


def kernel(x, task_ids, task_emb, Wp, bp, centers, A, Bm, adapter_scale):
    global _COMPILED, LAST_RESULT
    from concourse import bass_utils

    x = np.asarray(x, dtype=np.float32)
    task_ids = np.asarray(task_ids)
    task_emb = np.asarray(task_emb, dtype=np.float32)
    Wp = np.asarray(Wp, dtype=np.float32)
    bp = np.asarray(bp, dtype=np.float32)
    centers = np.asarray(centers, dtype=np.float32)
    A = np.asarray(A, dtype=np.float32)
    Bm = np.asarray(Bm, dtype=np.float32)

    if _COMPILED is None:
        _COMPILED = _build()
    nc = _COMPILED

    ah, wsn, bpk, e_np, biases = _prep_consts(
        task_emb, task_ids, Wp, bp, centers, A, Bm, adapter_scale)

    xf = x.reshape(B * S, H)
    xf8 = xf.astype(ml_dtypes.float8_e4m3)
    in_maps = []
    for c in range(NCORES):
        xtc = xf8[c * TPC:(c + 1) * TPC].reshape(NMT, T, NKT, 128)
        xtc = np.ascontiguousarray(xtc.transpose(3, 0, 2, 1)) \
            .reshape(128, NMT * NKT * T)
        in_maps.append({"xt": xtc, "ah": ah, "ws": wsn, "bpk": bpk,
                       "e": e_np, "bias": biases[c]})

    kwargs = {}
    if TRACE:
        kwargs = dict(trace=True, tmpdir=TRACE_DIR)
    res = bass_utils.run_bass_kernel_spmd(
        nc, in_maps, core_ids=list(range(NCORES)), **kwargs)
    LAST_RESULT = res

    out = np.empty((B * S, H), np.float32)
    for c in range(NCORES):
        dat = res.results[c]["yt"].reshape(128, NMT, 4, 8, T)
        delta = dat.transpose(1, 4, 2, 3, 0).reshape(TPC, H)
        out[c * TPC:(c + 1) * TPC] = xf[c * TPC:(c + 1) * TPC] + \
            delta.astype(np.float32)
    return out.reshape(B, S, H)


# revision 18
# speedup vs baseline: 1.1135x; 1.0899x over previous
"""Trainium2 Bass kernel for NeuroplasticLlama block-sparse adapter (moe_routing).

Contract: kernel(**inputs) takes FULL unsharded inputs (as produced by
setup_inputs) and returns the FULL [4, 4096, 4096] float32 output.

Strategy (data/sequence parallel over 8 cores, 2048 tokens each):
  - Each core's 2048 contiguous tokens belong to exactly one batch, so the
    task embedding contributes only per-core constant bias vectors
    (te @ A, te @ W2) -- h = x + te is never materialized.
  - Dense formulation of the routed computation:
      scores s[t,n] = x @ (Wp @ centers.T)[:,n] + const_n (shift dropped;
        top-k set and softmax are shift invariant)
      top-3 via threshold = 3rd max (3 rounds of max + mask-out)
      gates g[t,n] = exp(s - max) * (s >= thr3) / sum(...)
      z[t,:] = x @ A_all (all 512 block-rank pairs, dense)
      zg = z * expand4(g);  delta = block-diag(Bm) matmul
  - The device computes only DELTA (fp8 in, fp8 out); the residual
    y = x + delta is applied on the host during unsharding, keeping the
    x term exact and HBM traffic at 2 MB in + 2 MB out per 512-token
    macrotile per core.
  - x is fed pre-transposed and pre-shuffled to the [p][mt][k][t] layout
    so every DMA is a contiguous [128, N] transfer (128 descriptors; a
    strided [H, tokens] slice costs ~18us of descriptor generation).
  - scores and z are fp8 DoubleRow matmuls off the same x tile; delta is
    bf16 (output-stream bound, DoubleRow would not help).
  - SOFTWARE PIPELINE: macrotile m's delta/gate-apply phase executes
    during macrotile m+1's scores/z phase, with delta pair-matmuls
    interleaved between z chunks.  The PSUM->SBUF delta drains (the
    throughput-critical ~9us/mt of scalar+vector work) then overlap the
    z matmuls instead of bursting at the end of each macrotile, and the
    gating chain of m+1 runs in the vector queue after m's drains.
"""

import sys

if "/opt/trn_rl_repo" not in sys.path:
    sys.path.insert(0, "/opt/trn_rl_repo")

import numpy as np
import ml_dtypes

H = 4096
NB = 128
BLK = 32
R = 4
B = 4
S = 4096
NCORES = 8
TPC = (B * S) // NCORES  # tokens per core = 2048
T = 512                  # tokens per macrotile
NMT = TPC // T           # 4 macrotiles per core
NKT = H // 128           # 32 k-tiles over the hidden dim
NP = NKT // 2            # 16 DoubleRow k-pair tiles
BIG = 1.0e30

TRACE = False            # set by test.py for profiling runs
TRACE_DIR = None
LAST_RESULT = None       # BassKernelResults of the last run

_COMPILED = None


def _build():
    import concourse.bacc as bacc
    import concourse.tile as tile
    from concourse import mybir, masks

    f32 = mybir.dt.float32
    bf16 = mybir.dt.bfloat16
    f8 = mybir.dt.float8e4
    AF = mybir.ActivationFunctionType
    AL = mybir.AluOpType
    AX = mybir.AxisListType
    DR = mybir.MatmulPerfMode.DoubleRow

    nc = bacc.Bacc("TRN2", target_bir_lowering=False, debug=False,
                   num_devices=NCORES)

    xt_d = nc.dram_tensor("xt", [128, NMT * NKT * T], f8, kind="ExternalInput")
    ah_d = nc.dram_tensor("ah", [128, 4 * NKT * 128], f8, kind="ExternalInput")
    ws_d = nc.dram_tensor("ws", [128, NKT * 128], f8, kind="ExternalInput")
    bpk_d = nc.dram_tensor("bpk", [128, NKT * 128], bf16, kind="ExternalInput")
    e_d = nc.dram_tensor("e", [128, 512], bf16, kind="ExternalInput")
    bias_d = nc.dram_tensor("bias", [128, 5], f32, kind="ExternalInput")
    yt_d = nc.dram_tensor("yt", [128, NMT * NKT * T], f8, kind="ExternalOutput")

    xt_ap = xt_d.ap()
    yt_ap = yt_d.ap()

    with tile.TileContext(nc) as tc:
        from contextlib import ExitStack
        with ExitStack() as ctx:
            cpool = ctx.enter_context(tc.tile_pool(name="consts", bufs=1))
            xpool = ctx.enter_context(tc.tile_pool(name="xg", bufs=2))
            dpool = ctx.enter_context(tc.tile_pool(name="dall", bufs=2))
            zpool = ctx.enter_context(tc.tile_pool(name="zb", bufs=8))
            gpool = ctx.enter_context(tc.tile_pool(name="gate", bufs=3))
            spool = ctx.enter_context(tc.tile_pool(name="scal", bufs=4))
            pp = ctx.enter_context(tc.tile_pool(name="ps", bufs=2, space="PSUM"))

            # ---- persistent constants ----
            # xa0 is issued FIRST (in the macrotile loop below) on the sync
            # HWDGE ring; az/bpk/esb queue BEHIND it on the same ring so the
            # first scores matmul's input gets the full HBM bandwidth
            # (per-queue FIFO), while the tiny ws/bias go in parallel on
            # scalar/gpsimd.  az0 is only needed ~4us after scores start.
            ws = cpool.tile([128, NKT * 128], f8, name="ws", tag="ws")
            nc.scalar.dma_start(ws[:], ws_d.ap()[:])
            bias = cpool.tile([128, 5], f32, name="bias", tag="bias")
            nc.gpsimd.dma_start(bias[:], bias_d.ap()[:])
            ident = cpool.tile([128, 128], bf16, name="ident", tag="ident")
            masks.make_identity(nc, ident[:])
            kshift = cpool.tile([128, 1], f32, name="kshift", tag="kshift")
            nc.gpsimd.memset(kshift[:], 1000.0)
            az = []
            bpk = cpool.tile([128, NKT * 128], bf16, name="bpk", tag="bpk")
            esb = cpool.tile([128, 512], bf16, name="esb", tag="esb")

            def emit_const_loads():
                for q in range(4):
                    t_az = cpool.tile([128, NKT * 128], f8, name=f"az{q}",
                                      tag=f"az{q}")
                    nc.sync.dma_start(
                        t_az[:],
                        ah_d.ap()[:, q * NKT * 128:(q + 1) * NKT * 128])
                    az.append(t_az)
                nc.sync.dma_start(bpk[:], bpk_d.ap()[:])
                nc.sync.dma_start(esb[:], e_d.ap()[:])

            NTS = T // 128  # token sub-tiles per macrotile

            # ---------------- pipelined stage helpers ----------------
            def emit_scores(xa):
                sp = pp.tile([128, T], f32, space="PSUM", name="sp", tag="zp")
                for k2 in range(NP):
                    nc.tensor.matmul(
                        sp[:],
                        ws[:, k2 * 256:(k2 + 1) * 256]
                        .rearrange("p (two m) -> p two m", two=2),
                        xa[:, 2 * k2 * T:(2 * k2 + 2) * T]
                        .rearrange("p (two t) -> p two t", two=2),
                        start=(k2 == 0), stop=(k2 == NP - 1),
                        perf_mode=DR,
                    )
                s_sb = gpool.tile([128, T], bf16, name="s_sb", tag="s_sb")
                nc.scalar.activation(s_sb[:], sp[:], AF.Identity,
                                     bias=bias[:, 4:5], scale=1.0)
                return s_sb

            def emit_transpose_scores(s_sb):
                s_ps = pp.tile([128, T], bf16, space="PSUM", name="s_ps",
                               tag="tr", bufs=1)
                for ts in range(NTS):
                    nc.tensor.transpose(s_ps[:, ts * 128:(ts + 1) * 128],
                                        s_sb[:, ts * 128:(ts + 1) * 128],
                                        ident[:])
                stn_all = gpool.tile([128, T], f32, name="stn_all", tag="stn",
                                     bufs=2)
                # +1000 shift makes every score positive, so the chain can
                # mask out maxima with s*(s<r) in one fused op per round
                nc.scalar.activation(stn_all[:], s_ps[:], AF.Identity,
                                     bias=kshift[:], scale=1.0)
                return stn_all

            def emit_transpose_gates(ggs):
                g_ps = pp.tile([128, T], bf16, space="PSUM", name="g_ps",
                               tag="tr", bufs=1)
                for ts in range(NTS):
                    nc.tensor.transpose(g_ps[:, ts * 128:(ts + 1) * 128],
                                        ggs[ts][:], ident[:])
                gt_sb = gpool.tile([128, T], bf16, name="gt_sb", tag="gt_sb")
                nc.scalar.copy(gt_sb[:], g_ps[:])
                return gt_sb

            def emit_z_half(xa, q, half, zp):
                for k2 in range(NP // 2 * half, NP // 2 * (half + 1)):
                    nc.tensor.matmul(
                        zp[:],
                        az[q][:, k2 * 256:(k2 + 1) * 256]
                        .rearrange("p (two m) -> p two m", two=2),
                        xa[:, 2 * k2 * T:(2 * k2 + 2) * T]
                        .rearrange("p (two t) -> p two t", two=2),
                        start=(k2 == 0), stop=(k2 == NP - 1),
                        perf_mode=DR,
                    )

            def emit_gx_mul(st, q):
                gx = pp.tile([128, T], f32, space="PSUM", name="gx", tag="gx",
                             bufs=1)
                nc.tensor.matmul(gx[:],
                                 esb[:, q * 128:(q + 1) * 128],
                                 st["gt"][:],
                                 start=True, stop=True)
                nc.vector.tensor_mul(st["zbs"][q][:], st["zbs"][q][:], gx[:])

            def emit_delta_pairs(st, q, prs, veceng):
                # delta pair matmuls + PSUM->SBUF fp8 drains for pair
                # indices prs of quarter q of the previous macrotile
                for pr in prs:
                    hc = q * 8 + pr * 2
                    dp = pp.tile([128, 2 * T], f32, space="PSUM", name="dp",
                                 tag="dp", bufs=2)
                    for hf in range(2):
                        nc.tensor.matmul(
                            dp[:, hf * T:(hf + 1) * T],
                            bpk[:, (hc + hf) * 128:(hc + hf + 1) * 128],
                            st["zbs"][q][:],
                            start=True, stop=True)
                    if veceng == "both":
                        # tail drain: no later z-phase to hide behind, so
                        # both engines drain one half of every pair
                        nc.scalar.copy(st["da"][:, hc * T:(hc + 1) * T],
                                       dp[:, 0:T])
                        nc.vector.tensor_copy(
                            st["da"][:, (hc + 1) * T:(hc + 2) * T],
                            dp[:, T:2 * T])
                    elif pr in veceng:
                        nc.vector.tensor_copy(st["da"][:, hc * T:(hc + 2) * T],
                                              dp[:])
                    else:
                        nc.scalar.copy(st["da"][:, hc * T:(hc + 2) * T],
                                       dp[:])

            def emit_store(st, q):
                smt = st["mt"]
                nc.sync.dma_start(
                    yt_ap[:, (smt * 4 + q) * 8 * T:(smt * 4 + q + 1) * 8 * T],
                    st["da"][:, q * 8 * T:(q + 1) * 8 * T],
                )

            def emit_chain(stn_all):
                # the DVE gating chain for all 4 token sub-tiles
                ggs = []
                for ts in range(NTS):
                    stn = stn_all[:, ts * 128:(ts + 1) * 128]
                    r1 = spool.tile([128, 1], f32, name="r1", tag="r1")
                    nc.vector.reduce_max(r1[:], stn, axis=AX.X)
                    s2 = gpool.tile([128, 128], f32, name="s2", tag="s2")
                    nc.vector.scalar_tensor_tensor(s2[:], stn, r1[:], stn,
                                                   AL.is_lt, AL.mult)
                    r2 = spool.tile([128, 1], f32, name="r2", tag="r2")
                    nc.vector.reduce_max(r2[:], s2[:], axis=AX.X)
                    s3 = gpool.tile([128, 128], f32, name="s3", tag="s3")
                    nc.vector.scalar_tensor_tensor(s3[:], s2[:], r2[:], s2[:],
                                                   AL.is_lt, AL.mult)
                    r3 = spool.tile([128, 1], f32, name="r3", tag="r3")
                    nc.vector.reduce_max(r3[:], s3[:], axis=AX.X)
                    nr1 = spool.tile([128, 1], f32, name="nr1", tag="nr1")
                    nc.vector.tensor_scalar_mul(nr1[:], r1[:], -1.0)
                    ex = gpool.tile([128, 128], f32, name="ex", tag="ex")
                    nc.scalar.activation(ex[:], stn, AF.Exp, bias=nr1[:],
                                         scale=1.0)
                    em = gpool.tile([128, 128], f32, name="em", tag="em")
                    zs = spool.tile([128, 1], f32, name="zs", tag="zs")
                    nc.vector.scalar_tensor_tensor(em[:], stn, r3[:], ex[:],
                                                   AL.is_ge, AL.mult,
                                                   accum_out=zs[:])
                    rz = spool.tile([128, 1], f32, name="rz", tag="rz")
                    nc.vector.reciprocal(rz[:], zs[:])
                    gg = gpool.tile([128, 128], bf16, name="gg", tag="gg",
                                    bufs=NTS + 1)
                    nc.vector.tensor_scalar_mul(gg[:], em[:], rz[:])
                    ggs.append(gg)
                return ggs

            # ---------------- the pipelined macrotile loop ----------------
            prev = None  # state of the previous macrotile
            for mt in range(NMT):
                xa = xpool.tile([128, NKT * T], f8, name="xa", tag="xa")
                PART = NKT * T // 4
                for part in range(4):
                    nc.sync.dma_start(
                        xa[:, part * PART:(part + 1) * PART],
                        xt_ap[:, mt * NKT * T + part * PART:
                              mt * NKT * T + (part + 1) * PART])
                if mt == 0:
                    emit_const_loads()

                s_sb = emit_scores(xa)
                stn_all = emit_transpose_scores(s_sb)
                if prev is not None:
                    prev["gt"] = emit_transpose_gates(prev["ggs"])

                da = dpool.tile([128, NKT * T], f8, name="da", tag="da")
                cur = {"mt": mt, "da": da, "zbs": [], "ggs": None}

                for q in range(4):
                    if prev is not None:
                        emit_gx_mul(prev, q)
                        emit_delta_pairs(prev, q, (0, 1), veceng={1})
                    zp = pp.tile([128, T], f32, space="PSUM", name="zp",
                                 tag="zp")
                    emit_z_half(xa, q, 0, zp)
                    if prev is not None:
                        emit_delta_pairs(prev, q, (2, 3),
                                         veceng={3} if q % 2 else set())
                    emit_z_half(xa, q, 1, zp)
                    if prev is not None:
                        emit_store(prev, q)
                    zb = zpool.tile([128, T], bf16, name="zb", tag="zb")
                    nc.scalar.activation(zb[:], zp[:], AF.Identity,
                                         bias=bias[:, q:q + 1], scale=1.0)
                    cur["zbs"].append(zb)

                cur["ggs"] = emit_chain(stn_all)
                prev = cur

            # ---- drain the pipeline: delta of the last macrotile ----
            prev["gt"] = emit_transpose_gates(prev["ggs"])
            for q in range(4):
                emit_gx_mul(prev, q)
                emit_delta_pairs(prev, q, (0, 1, 2, 3), veceng="both")
                emit_store(prev, q)

    nc.compile()
    return nc


def _prep_consts(task_emb, task_ids, Wp, bp, centers, A, Bm, adapter_scale):
    scale = float(np.asarray(adapter_scale))
    A_all = np.ascontiguousarray(
        A.transpose(1, 0, 2).reshape(H, NB * R).astype(np.float32))
    W2 = (Wp @ centers.T).astype(np.float32)                     # [H, 128]

    # ah: [p, q, k2, two, m] = A_all[(2*k2+two)*128+p, q*128+m], fp8 e4m3
    # (DoubleRow pairs of consecutive k-tiles interleave along the free dim)
    ah = (A_all.reshape(NKT, 128, 4, 128).transpose(1, 2, 0, 3)
          .reshape(128, 4 * NKT * 128).astype(ml_dtypes.float8_e4m3))
    ah = np.ascontiguousarray(ah)
    # ws: [p, k2, two, m] = W2[(2*k2+two)*128+p, m], fp8 (DoubleRow pairs)
    wsn = np.ascontiguousarray(
        W2.reshape(NKT, 128, 128).transpose(1, 0, 2).reshape(128, NKT * 128)
        .astype(ml_dtypes.float8_e4m3))

    # block-diag up-projection, K=128 per h-chunk
    bpk = np.zeros((128, NKT * 128), np.float32)
    for hc in range(NKT):
        for mblk in range(4):
            n = hc * 4 + mblk
            for r in range(R):
                row = (hc % 8) * 16 + mblk * 4 + r
                bpk[row, hc * 128 + mblk * 32: hc * 128 + mblk * 32 + 32] = \
                    Bm[n, r, :] * scale
    bpk = bpk.astype(ml_dtypes.bfloat16)

    e_np = (np.arange(128)[:, None] == (np.arange(512)[None, :] // 4)) \
        .astype(ml_dtypes.bfloat16)

    sconst = (bp @ centers.T - 0.5 * (centers ** 2).sum(-1)).astype(np.float32)

    biases = []
    for c in range(NCORES):
        te = task_emb[int(np.asarray(task_ids)[c // 2])].astype(np.float32)
        b5 = np.empty((128, 5), np.float32)
        zoff = te @ A_all                                        # [512]
        for q in range(4):
            b5[:, q] = zoff[q * 128:(q + 1) * 128]
        b5[:, 4] = te @ W2 + sconst
        biases.append(np.ascontiguousarray(b5))
    return ah, wsn, bpk, e_np, biases


def kernel(x, task_ids, task_emb, Wp, bp, centers, A, Bm, adapter_scale):
    global _COMPILED, LAST_RESULT
    from concourse import bass_utils

    x = np.asarray(x, dtype=np.float32)
    task_ids = np.asarray(task_ids)
    task_emb = np.asarray(task_emb, dtype=np.float32)
    Wp = np.asarray(Wp, dtype=np.float32)
    bp = np.asarray(bp, dtype=np.float32)
    centers = np.asarray(centers, dtype=np.float32)
    A = np.asarray(A, dtype=np.float32)
    Bm = np.asarray(Bm, dtype=np.float32)

    if _COMPILED is None:
        _COMPILED = _build()
    nc = _COMPILED

    ah, wsn, bpk, e_np, biases = _prep_consts(
        task_emb, task_ids, Wp, bp, centers, A, Bm, adapter_scale)

    xf = x.reshape(B * S, H)
    xf8 = xf.astype(ml_dtypes.float8_e4m3)
    in_maps = []
    for c in range(NCORES):
        xtc = xf8[c * TPC:(c + 1) * TPC].reshape(NMT, T, NKT, 128)
        xtc = np.ascontiguousarray(xtc.transpose(3, 0, 2, 1)) \
            .reshape(128, NMT * NKT * T)
        in_maps.append({"xt": xtc, "ah": ah, "ws": wsn, "bpk": bpk,
                       "e": e_np, "bias": biases[c]})

    kwargs = {}
    if TRACE:
        kwargs = dict(trace=True, tmpdir=TRACE_DIR)
    res = bass_utils.run_bass_kernel_spmd(
        nc, in_maps, core_ids=list(range(NCORES)), **kwargs)
    LAST_RESULT = res

    out = np.empty((B * S, H), np.float32)
    for c in range(NCORES):
        dat = res.results[c]["yt"].reshape(128, NMT, 4, 8, T)
        delta = dat.transpose(1, 4, 2, 3, 0).reshape(TPC, H)
        out[c * TPC:(c + 1) * TPC] = xf[c * TPC:(c + 1) * TPC] + \
            delta.astype(np.float32)
    return out.reshape(B, S, H)


# revision 19
# speedup vs baseline: 1.1210x; 1.0068x over previous
"""Trainium2 Bass kernel for NeuroplasticLlama block-sparse adapter (moe_routing).

Contract: kernel(**inputs) takes FULL unsharded inputs (as produced by
setup_inputs) and returns the FULL [4, 4096, 4096] float32 output.

Strategy (data/sequence parallel over 8 cores, 2048 tokens each):
  - Each core's 2048 contiguous tokens belong to exactly one batch, so the
    task embedding contributes only per-core constant bias vectors
    (te @ A, te @ W2) -- h = x + te is never materialized.
  - Dense formulation of the routed computation:
      scores s[t,n] = x @ (Wp @ centers.T)[:,n] + const_n (shift dropped;
        top-k set and softmax are shift invariant)
      top-3 via threshold = 3rd max (3 rounds of max + mask-out)
      gates g[t,n] = exp(s - max) * (s >= thr3) / sum(...)
      z[t,:] = x @ A_all (all 512 block-rank pairs, dense)
      zg = z * expand4(g);  delta = block-diag(Bm) matmul
  - The device computes only DELTA (fp8 in, fp8 out); the residual
    y = x + delta is applied on the host during unsharding, keeping the
    x term exact and HBM traffic at 2 MB in + 2 MB out per 512-token
    macrotile per core.
  - x is fed pre-transposed and pre-shuffled to the [p][mt][k][t] layout
    so every DMA is a contiguous [128, N] transfer (128 descriptors; a
    strided [H, tokens] slice costs ~18us of descriptor generation).
  - scores and z are fp8 DoubleRow matmuls off the same x tile; delta is
    bf16 (output-stream bound, DoubleRow would not help).
  - SOFTWARE PIPELINE: macrotile m's delta/gate-apply phase executes
    during macrotile m+1's scores/z phase, with delta pair-matmuls
    interleaved between z chunks.  The PSUM->SBUF delta drains (the
    throughput-critical ~9us/mt of scalar+vector work) then overlap the
    z matmuls instead of bursting at the end of each macrotile, and the
    gating chain of m+1 runs in the vector queue after m's drains.
"""

import sys

if "/opt/trn_rl_repo" not in sys.path:
    sys.path.insert(0, "/opt/trn_rl_repo")

import numpy as np
import ml_dtypes

H = 4096
NB = 128
BLK = 32
R = 4
B = 4
S = 4096
NCORES = 8
TPC = (B * S) // NCORES  # tokens per core = 2048
T = 512                  # tokens per macrotile
NMT = TPC // T           # 4 macrotiles per core
NKT = H // 128           # 32 k-tiles over the hidden dim
NP = NKT // 2            # 16 DoubleRow k-pair tiles
BIG = 1.0e30

TRACE = False            # set by test.py for profiling runs
TRACE_DIR = None
LAST_RESULT = None       # BassKernelResults of the last run

_COMPILED = None


def _build():
    import concourse.bacc as bacc
    import concourse.tile as tile
    from concourse import mybir, masks

    f32 = mybir.dt.float32
    bf16 = mybir.dt.bfloat16
    f8 = mybir.dt.float8e4
    AF = mybir.ActivationFunctionType
    AL = mybir.AluOpType
    AX = mybir.AxisListType
    DR = mybir.MatmulPerfMode.DoubleRow

    nc = bacc.Bacc("TRN2", target_bir_lowering=False, debug=False,
                   num_devices=NCORES)

    xt_d = nc.dram_tensor("xt", [128, NMT * NKT * T], f8, kind="ExternalInput")
    ah_d = nc.dram_tensor("ah", [128, 4 * NKT * 128], f8, kind="ExternalInput")
    ws_d = nc.dram_tensor("ws", [128, NKT * 128], f8, kind="ExternalInput")
    bpk_d = nc.dram_tensor("bpk", [128, NKT * 128], bf16, kind="ExternalInput")
    e_d = nc.dram_tensor("e", [128, 512], bf16, kind="ExternalInput")
    bias_d = nc.dram_tensor("bias", [128, 5], f32, kind="ExternalInput")
    yt_d = nc.dram_tensor("yt", [128, NMT * NKT * T], f8, kind="ExternalOutput")

    xt_ap = xt_d.ap()
    yt_ap = yt_d.ap()

    with tile.TileContext(nc) as tc:
        from contextlib import ExitStack
        with ExitStack() as ctx:
            cpool = ctx.enter_context(tc.tile_pool(name="consts", bufs=1))
            xpool = ctx.enter_context(tc.tile_pool(name="xg", bufs=2))
            dpool = ctx.enter_context(tc.tile_pool(name="dall", bufs=2))
            zpool = ctx.enter_context(tc.tile_pool(name="zb", bufs=8))
            gpool = ctx.enter_context(tc.tile_pool(name="gate", bufs=3))
            spool = ctx.enter_context(tc.tile_pool(name="scal", bufs=4))
            pp = ctx.enter_context(tc.tile_pool(name="ps", bufs=2, space="PSUM"))

            # ---- persistent constants ----
            # xa0 is issued FIRST (in the macrotile loop below) on the sync
            # HWDGE ring; az/bpk/esb queue BEHIND it on the same ring so the
            # first scores matmul's input gets the full HBM bandwidth
            # (per-queue FIFO), while the tiny ws/bias go in parallel on
            # scalar/gpsimd.  az0 is only needed ~4us after scores start.
            ws = cpool.tile([128, NKT * 128], f8, name="ws", tag="ws")
            nc.scalar.dma_start(ws[:], ws_d.ap()[:])
            bias = cpool.tile([128, 5], f32, name="bias", tag="bias")
            nc.gpsimd.dma_start(bias[:], bias_d.ap()[:])
            ident = cpool.tile([128, 128], bf16, name="ident", tag="ident")
            masks.make_identity(nc, ident[:])
            kshift = cpool.tile([128, 1], f32, name="kshift", tag="kshift")
            nc.gpsimd.memset(kshift[:], 1000.0)
            az = []
            bpk = cpool.tile([128, NKT * 128], bf16, name="bpk", tag="bpk")
            esb = cpool.tile([128, 512], bf16, name="esb", tag="esb")

            def emit_const_loads():
                for q in range(4):
                    t_az = cpool.tile([128, NKT * 128], f8, name=f"az{q}",
                                      tag=f"az{q}")
                    nc.sync.dma_start(
                        t_az[:],
                        ah_d.ap()[:, q * NKT * 128:(q + 1) * NKT * 128])
                    az.append(t_az)
                nc.sync.dma_start(bpk[:], bpk_d.ap()[:])
                nc.sync.dma_start(esb[:], e_d.ap()[:])

            NTS = T // 128  # token sub-tiles per macrotile

            # ---------------- pipelined stage helpers ----------------
            def emit_scores(xa):
                sp = pp.tile([128, T], f32, space="PSUM", name="sp", tag="zp")
                for k2 in range(NP):
                    nc.tensor.matmul(
                        sp[:],
                        ws[:, k2 * 256:(k2 + 1) * 256]
                        .rearrange("p (two m) -> p two m", two=2),
                        xa[:, 2 * k2 * T:(2 * k2 + 2) * T]
                        .rearrange("p (two t) -> p two t", two=2),
                        start=(k2 == 0), stop=(k2 == NP - 1),
                        perf_mode=DR,
                    )
                s_sb = gpool.tile([128, T], bf16, name="s_sb", tag="s_sb")
                nc.scalar.activation(s_sb[:], sp[:], AF.Identity,
                                     bias=bias[:, 4:5], scale=1.0)
                return s_sb

            def emit_transpose_scores(s_sb):
                s_ps = pp.tile([128, T], bf16, space="PSUM", name="s_ps",
                               tag="tr", bufs=1)
                for ts in range(NTS):
                    nc.tensor.transpose(s_ps[:, ts * 128:(ts + 1) * 128],
                                        s_sb[:, ts * 128:(ts + 1) * 128],
                                        ident[:])
                stn_all = gpool.tile([128, T], f32, name="stn_all", tag="stn",
                                     bufs=2)
                # +1000 shift makes every score positive, so the chain can
                # mask out maxima with s*(s<r) in one fused op per round
                nc.scalar.activation(stn_all[:], s_ps[:], AF.Identity,
                                     bias=kshift[:], scale=1.0)
                return stn_all

            def emit_transpose_gates(ggs):
                g_ps = pp.tile([128, T], bf16, space="PSUM", name="g_ps",
                               tag="tr", bufs=1)
                for ts in range(NTS):
                    nc.tensor.transpose(g_ps[:, ts * 128:(ts + 1) * 128],
                                        ggs[ts][:], ident[:])
                gt_sb = gpool.tile([128, T], bf16, name="gt_sb", tag="gt_sb")
                nc.scalar.copy(gt_sb[:], g_ps[:])
                return gt_sb

            def emit_z_half(xa, q, half, zp):
                for k2 in range(NP // 2 * half, NP // 2 * (half + 1)):
                    nc.tensor.matmul(
                        zp[:],
                        az[q][:, k2 * 256:(k2 + 1) * 256]
                        .rearrange("p (two m) -> p two m", two=2),
                        xa[:, 2 * k2 * T:(2 * k2 + 2) * T]
                        .rearrange("p (two t) -> p two t", two=2),
                        start=(k2 == 0), stop=(k2 == NP - 1),
                        perf_mode=DR,
                    )

            def emit_gx_mul(st, q):
                gx = pp.tile([128, T], f32, space="PSUM", name="gx", tag="gx",
                             bufs=1)
                nc.tensor.matmul(gx[:],
                                 esb[:, q * 128:(q + 1) * 128],
                                 st["gt"][:],
                                 start=True, stop=True)
                nc.vector.tensor_mul(st["zbs"][q][:], st["zbs"][q][:], gx[:])

            def emit_delta_pairs(st, q, prs, veceng):
                # delta pair matmuls + PSUM->SBUF fp8 drains for pair
                # indices prs of quarter q of the previous macrotile
                for pr in prs:
                    hc = q * 8 + pr * 2
                    dp = pp.tile([128, 2 * T], f32, space="PSUM", name="dp",
                                 tag="dp", bufs=2)
                    for hf in range(2):
                        nc.tensor.matmul(
                            dp[:, hf * T:(hf + 1) * T],
                            bpk[:, (hc + hf) * 128:(hc + hf + 1) * 128],
                            st["zbs"][q][:],
                            start=True, stop=True)
                    if veceng == "both":
                        # tail drain: no later z-phase to hide behind, so
                        # both engines drain one half of every pair
                        nc.scalar.copy(st["da"][:, hc * T:(hc + 1) * T],
                                       dp[:, 0:T])
                        nc.vector.tensor_copy(
                            st["da"][:, (hc + 1) * T:(hc + 2) * T],
                            dp[:, T:2 * T])
                    elif pr in veceng:
                        nc.vector.tensor_copy(st["da"][:, hc * T:(hc + 2) * T],
                                              dp[:])
                    else:
                        nc.scalar.copy(st["da"][:, hc * T:(hc + 2) * T],
                                       dp[:])

            def emit_store(st, q):
                smt = st["mt"]
                nc.sync.dma_start(
                    yt_ap[:, (smt * 4 + q) * 8 * T:(smt * 4 + q + 1) * 8 * T],
                    st["da"][:, q * 8 * T:(q + 1) * 8 * T],
                )

            def emit_chain(stn_all):
                # the DVE gating chain for all 4 token sub-tiles
                ggs = []
                for ts in range(NTS):
                    stn = stn_all[:, ts * 128:(ts + 1) * 128]
                    r1 = spool.tile([128, 1], f32, name="r1", tag="r1")
                    nc.vector.reduce_max(r1[:], stn, axis=AX.X)
                    s2 = gpool.tile([128, 128], f32, name="s2", tag="s2")
                    nc.vector.scalar_tensor_tensor(s2[:], stn, r1[:], stn,
                                                   AL.is_lt, AL.mult)
                    r2 = spool.tile([128, 1], f32, name="r2", tag="r2")
                    nc.vector.reduce_max(r2[:], s2[:], axis=AX.X)
                    s3 = gpool.tile([128, 128], f32, name="s3", tag="s3")
                    nc.vector.scalar_tensor_tensor(s3[:], s2[:], r2[:], s2[:],
                                                   AL.is_lt, AL.mult)
                    r3 = spool.tile([128, 1], f32, name="r3", tag="r3")
                    nc.vector.reduce_max(r3[:], s3[:], axis=AX.X)
                    nr1 = spool.tile([128, 1], f32, name="nr1", tag="nr1")
                    nc.vector.tensor_scalar_mul(nr1[:], r1[:], -1.0)
                    ex = gpool.tile([128, 128], f32, name="ex", tag="ex")
                    nc.scalar.activation(ex[:], stn, AF.Exp, bias=nr1[:],
                                         scale=1.0)
                    em = gpool.tile([128, 128], f32, name="em", tag="em")
                    zs = spool.tile([128, 1], f32, name="zs", tag="zs")
                    nc.vector.scalar_tensor_tensor(em[:], stn, r3[:], ex[:],
                                                   AL.is_ge, AL.mult,
                                                   accum_out=zs[:])
                    rz = spool.tile([128, 1], f32, name="rz", tag="rz")
                    nc.vector.reciprocal(rz[:], zs[:])
                    gg = gpool.tile([128, 128], bf16, name="gg", tag="gg",
                                    bufs=NTS + 1)
                    nc.vector.tensor_scalar_mul(gg[:], em[:], rz[:])
                    ggs.append(gg)
                return ggs

            # ---------------- the pipelined macrotile loop ----------------
            prev = None  # state of the previous macrotile
            for mt in range(NMT):
                xa = xpool.tile([128, NKT * T], f8, name="xa", tag="xa")
                PART = NKT * T // 4
                for part in range(4):
                    nc.sync.dma_start(
                        xa[:, part * PART:(part + 1) * PART],
                        xt_ap[:, mt * NKT * T + part * PART:
                              mt * NKT * T + (part + 1) * PART])
                if mt == 0:
                    emit_const_loads()

                s_sb = emit_scores(xa)
                stn_all = emit_transpose_scores(s_sb)
                if prev is not None:
                    prev["gt"] = emit_transpose_gates(prev["ggs"])

                da = dpool.tile([128, NKT * T], f8, name="da", tag="da")
                cur = {"mt": mt, "da": da, "zbs": [], "ggs": None}

                for q in range(4):
                    if prev is not None:
                        emit_gx_mul(prev, q)
                        emit_delta_pairs(prev, q, (0, 1), veceng={1})
                    zp = pp.tile([128, T], f32, space="PSUM", name="zp",
                                 tag="zp")
                    emit_z_half(xa, q, 0, zp)
                    if prev is not None:
                        emit_delta_pairs(prev, q, (2, 3),
                                         veceng={3} if q % 2 else set())
                    emit_z_half(xa, q, 1, zp)
                    if prev is not None:
                        emit_store(prev, q)
                    zb = zpool.tile([128, T], bf16, name="zb", tag="zb")
                    nc.scalar.activation(zb[:], zp[:], AF.Identity,
                                         bias=bias[:, q:q + 1], scale=1.0)
                    cur["zbs"].append(zb)

                cur["ggs"] = emit_chain(stn_all)
                prev = cur

            # ---- drain the pipeline: delta of the last macrotile ----
            prev["gt"] = emit_transpose_gates(prev["ggs"])
            # tail drain: the z-phase "zp" PSUM tiles are dead now, so the
            # delta chunks cycle through dp pairs AND zp singles (6 chunks
            # in flight instead of 4), every pair drained by both engines
            for q in range(4):
                emit_gx_mul(prev, q)
                zgq = prev["zbs"][q]
                for grp in range(2):           # chunks [0..3] then [4..7]
                    hc0 = q * 8 + grp * 4
                    dp = pp.tile([128, 2 * T], f32, space="PSUM", name="dp",
                                 tag="dp", bufs=2)
                    for hf in range(2):
                        nc.tensor.matmul(
                            dp[:, hf * T:(hf + 1) * T],
                            bpk[:, (hc0 + hf) * 128:(hc0 + hf + 1) * 128],
                            zgq[:], start=True, stop=True)
                    nc.scalar.copy(prev["da"][:, hc0 * T:(hc0 + 1) * T],
                                   dp[:, 0:T])
                    nc.vector.tensor_copy(
                        prev["da"][:, (hc0 + 1) * T:(hc0 + 2) * T],
                        dp[:, T:2 * T])
                    for sng in range(2):
                        hc = hc0 + 2 + sng
                        zps = pp.tile([128, T], f32, space="PSUM", name="zp",
                                      tag="zp")
                        nc.tensor.matmul(zps[:],
                                         bpk[:, hc * 128:(hc + 1) * 128],
                                         zgq[:], start=True, stop=True)
                        dsl = prev["da"][:, hc * T:(hc + 1) * T]
                        if sng == 0:
                            nc.scalar.copy(dsl, zps[:])
                        else:
                            nc.vector.tensor_copy(dsl, zps[:])
                emit_store(prev, q)

    nc.compile()
    return nc


def _prep_consts(task_emb, task_ids, Wp, bp, centers, A, Bm, adapter_scale):
    scale = float(np.asarray(adapter_scale))
    A_all = np.ascontiguousarray(
        A.transpose(1, 0, 2).reshape(H, NB * R).astype(np.float32))
    W2 = (Wp @ centers.T).astype(np.float32)                     # [H, 128]

    # ah: [p, q, k2, two, m] = A_all[(2*k2+two)*128+p, q*128+m], fp8 e4m3
    # (DoubleRow pairs of consecutive k-tiles interleave along the free dim)
    ah = (A_all.reshape(NKT, 128, 4, 128).transpose(1, 2, 0, 3)
          .reshape(128, 4 * NKT * 128).astype(ml_dtypes.float8_e4m3))
    ah = np.ascontiguousarray(ah)
    # ws: [p, k2, two, m] = W2[(2*k2+two)*128+p, m], fp8 (DoubleRow pairs)
    wsn = np.ascontiguousarray(
        W2.reshape(NKT, 128, 128).transpose(1, 0, 2).reshape(128, NKT * 128)
        .astype(ml_dtypes.float8_e4m3))

    # block-diag up-projection, K=128 per h-chunk
    bpk = np.zeros((128, NKT * 128), np.float32)
    for hc in range(NKT):
        for mblk in range(4):
            n = hc * 4 + mblk
            for r in range(R):
                row = (hc % 8) * 16 + mblk * 4 + r
                bpk[row, hc * 128 + mblk * 32: hc * 128 + mblk * 32 + 32] = \
                    Bm[n, r, :] * scale
    bpk = bpk.astype(ml_dtypes.bfloat16)

    e_np = (np.arange(128)[:, None] == (np.arange(512)[None, :] // 4)) \
        .astype(ml_dtypes.bfloat16)

    sconst = (bp @ centers.T - 0.5 * (centers ** 2).sum(-1)).astype(np.float32)

    biases = []
    for c in range(NCORES):
        te = task_emb[int(np.asarray(task_ids)[c // 2])].astype(np.float32)
        b5 = np.empty((128, 5), np.float32)
        zoff = te @ A_all                                        # [512]
        for q in range(4):
            b5[:, q] = zoff[q * 128:(q + 1) * 128]
        b5[:, 4] = te @ W2 + sconst
        biases.append(np.ascontiguousarray(b5))
    return ah, wsn, bpk, e_np, biases


def kernel(x, task_ids, task_emb, Wp, bp, centers, A, Bm, adapter_scale):
    global _COMPILED, LAST_RESULT
    from concourse import bass_utils

    x = np.asarray(x, dtype=np.float32)
    task_ids = np.asarray(task_ids)
    task_emb = np.asarray(task_emb, dtype=np.float32)
    Wp = np.asarray(Wp, dtype=np.float32)
    bp = np.asarray(bp, dtype=np.float32)
    centers = np.asarray(centers, dtype=np.float32)
    A = np.asarray(A, dtype=np.float32)
    Bm = np.asarray(Bm, dtype=np.float32)

    if _COMPILED is None:
        _COMPILED = _build()
    nc = _COMPILED

    ah, wsn, bpk, e_np, biases = _prep_consts(
        task_emb, task_ids, Wp, bp, centers, A, Bm, adapter_scale)

    xf = x.reshape(B * S, H)
    xf8 = xf.astype(ml_dtypes.float8_e4m3)
    in_maps = []
    for c in range(NCORES):
        xtc = xf8[c * TPC:(c + 1) * TPC].reshape(NMT, T, NKT, 128)
        xtc = np.ascontiguousarray(xtc.transpose(3, 0, 2, 1)) \
            .reshape(128, NMT * NKT * T)
        in_maps.append({"xt": xtc, "ah": ah, "ws": wsn, "bpk": bpk,
                       "e": e_np, "bias": biases[c]})

    kwargs = {}
    if TRACE:
        kwargs = dict(trace=True, tmpdir=TRACE_DIR)
    res = bass_utils.run_bass_kernel_spmd(
        nc, in_maps, core_ids=list(range(NCORES)), **kwargs)
    LAST_RESULT = res

    out = np.empty((B * S, H), np.float32)
    for c in range(NCORES):
        dat = res.results[c]["yt"].reshape(128, NMT, 4, 8, T)
        delta = dat.transpose(1, 4, 2, 3, 0).reshape(TPC, H)
        out[c * TPC:(c + 1) * TPC] = xf[c * TPC:(c + 1) * TPC] + \
            delta.astype(np.float32)
    return out.reshape(B, S, H)


# revision 20
# speedup vs baseline: 1.1312x; 1.0090x over previous
"""Trainium2 Bass kernel for NeuroplasticLlama block-sparse adapter (moe_routing).

Contract: kernel(**inputs) takes FULL unsharded inputs (as produced by
setup_inputs) and returns the FULL [4, 4096, 4096] float32 output.

Strategy (data/sequence parallel over 8 cores, 2048 tokens each):
  - Each core's 2048 contiguous tokens belong to exactly one batch, so the
    task embedding contributes only per-core constant bias vectors
    (te @ A, te @ W2) -- h = x + te is never materialized.
  - Dense formulation of the routed computation:
      scores s[t,n] = x @ (Wp @ centers.T)[:,n] + const_n (shift dropped;
        top-k set and softmax are shift invariant)
      top-3 via threshold = 3rd max (3 rounds of max + mask-out)
      gates g[t,n] = exp(s - max) * (s >= thr3) / sum(...)
      z[t,:] = x @ A_all (all 512 block-rank pairs, dense)
      zg = z * expand4(g);  delta = block-diag(Bm) matmul
  - The device computes only DELTA (fp8 in, fp8 out); the residual
    y = x + delta is applied on the host during unsharding, keeping the
    x term exact and HBM traffic at 2 MB in + 2 MB out per 512-token
    macrotile per core.
  - x is fed pre-transposed and pre-shuffled to the [p][mt][k][t] layout
    so every DMA is a contiguous [128, N] transfer (128 descriptors; a
    strided [H, tokens] slice costs ~18us of descriptor generation).
  - scores and z are fp8 DoubleRow matmuls off the same x tile; delta is
    bf16 (output-stream bound, DoubleRow would not help).
  - SOFTWARE PIPELINE: macrotile m's delta/gate-apply phase executes
    during macrotile m+1's scores/z phase, with delta pair-matmuls
    interleaved between z chunks.  The PSUM->SBUF delta drains (the
    throughput-critical ~9us/mt of scalar+vector work) then overlap the
    z matmuls instead of bursting at the end of each macrotile, and the
    gating chain of m+1 runs in the vector queue after m's drains.
"""

import sys

if "/opt/trn_rl_repo" not in sys.path:
    sys.path.insert(0, "/opt/trn_rl_repo")

import numpy as np
import ml_dtypes

H = 4096
NB = 128
BLK = 32
R = 4
B = 4
S = 4096
NCORES = 8
TPC = (B * S) // NCORES  # tokens per core = 2048
T = 512                  # tokens per macrotile
NMT = TPC // T           # 4 macrotiles per core
NKT = H // 128           # 32 k-tiles over the hidden dim
NP = NKT // 2            # 16 DoubleRow k-pair tiles
BIG = 1.0e30

TRACE = False            # set by test.py for profiling runs
TRACE_DIR = None
LAST_RESULT = None       # BassKernelResults of the last run

_COMPILED = None


def _build():
    import concourse.bacc as bacc
    import concourse.tile as tile
    from concourse import mybir, masks

    f32 = mybir.dt.float32
    bf16 = mybir.dt.bfloat16
    f8 = mybir.dt.float8e4
    AF = mybir.ActivationFunctionType
    AL = mybir.AluOpType
    AX = mybir.AxisListType
    DR = mybir.MatmulPerfMode.DoubleRow

    nc = bacc.Bacc("TRN2", target_bir_lowering=False, debug=False,
                   num_devices=NCORES)

    xt_d = nc.dram_tensor("xt", [128, NMT * NKT * T], f8, kind="ExternalInput")
    ah_d = nc.dram_tensor("ah", [128, 4 * NKT * 128], f8, kind="ExternalInput")
    ws_d = nc.dram_tensor("ws", [128, NKT * 128], f8, kind="ExternalInput")
    bpk_d = nc.dram_tensor("bpk", [128, NKT * 128], bf16, kind="ExternalInput")
    e_d = nc.dram_tensor("e", [128, 512], bf16, kind="ExternalInput")
    bias_d = nc.dram_tensor("bias", [128, 5], f32, kind="ExternalInput")
    yt_d = nc.dram_tensor("yt", [128, NMT * NKT * T], f8, kind="ExternalOutput")

    xt_ap = xt_d.ap()
    yt_ap = yt_d.ap()

    with tile.TileContext(nc) as tc:
        from contextlib import ExitStack
        with ExitStack() as ctx:
            cpool = ctx.enter_context(tc.tile_pool(name="consts", bufs=1))
            xpool = ctx.enter_context(tc.tile_pool(name="xg", bufs=2))
            dpool = ctx.enter_context(tc.tile_pool(name="dall", bufs=2))
            zpool = ctx.enter_context(tc.tile_pool(name="zb", bufs=8))
            gpool = ctx.enter_context(tc.tile_pool(name="gate", bufs=3))
            spool = ctx.enter_context(tc.tile_pool(name="scal", bufs=4))
            pp = ctx.enter_context(tc.tile_pool(name="ps", bufs=2, space="PSUM"))

            # ---- persistent constants ----
            # xa0 is issued FIRST (in the macrotile loop below) on the sync
            # HWDGE ring; az/bpk/esb queue BEHIND it on the same ring so the
            # first scores matmul's input gets the full HBM bandwidth
            # (per-queue FIFO), while the tiny ws/bias go in parallel on
            # scalar/gpsimd.  az0 is only needed ~4us after scores start.
            ws = cpool.tile([128, NKT * 128], f8, name="ws", tag="ws")
            nc.scalar.dma_start(ws[:], ws_d.ap()[:])
            bias = cpool.tile([128, 5], f32, name="bias", tag="bias")
            nc.gpsimd.dma_start(bias[:], bias_d.ap()[:])
            ident = cpool.tile([128, 128], bf16, name="ident", tag="ident")
            masks.make_identity(nc, ident[:])
            kshift = cpool.tile([128, 1], f32, name="kshift", tag="kshift")
            nc.gpsimd.memset(kshift[:], 1000.0)
            az = []
            bpk = cpool.tile([128, NKT * 128], bf16, name="bpk", tag="bpk")
            esb = cpool.tile([128, 512], bf16, name="esb", tag="esb")

            def emit_const_loads():
                for q in range(4):
                    t_az = cpool.tile([128, NKT * 128], f8, name=f"az{q}",
                                      tag=f"az{q}")
                    nc.sync.dma_start(
                        t_az[:],
                        ah_d.ap()[:, q * NKT * 128:(q + 1) * NKT * 128])
                    az.append(t_az)
                nc.sync.dma_start(bpk[:], bpk_d.ap()[:])
                nc.sync.dma_start(esb[:], e_d.ap()[:])

            NTS = T // 128  # token sub-tiles per macrotile

            # ---------------- pipelined stage helpers ----------------
            def emit_scores(xa):
                sp = pp.tile([128, T], f32, space="PSUM", name="sp", tag="zp")
                for k2 in range(NP):
                    nc.tensor.matmul(
                        sp[:],
                        ws[:, k2 * 256:(k2 + 1) * 256]
                        .rearrange("p (two m) -> p two m", two=2),
                        xa[:, 2 * k2 * T:(2 * k2 + 2) * T]
                        .rearrange("p (two t) -> p two t", two=2),
                        start=(k2 == 0), stop=(k2 == NP - 1),
                        perf_mode=DR,
                    )
                s_sb = gpool.tile([128, T], bf16, name="s_sb", tag="s_sb")
                nc.scalar.activation(s_sb[:], sp[:], AF.Identity,
                                     bias=bias[:, 4:5], scale=1.0)
                return s_sb

            def emit_transpose_scores(s_sb):
                s_ps = pp.tile([128, T], bf16, space="PSUM", name="s_ps",
                               tag="tr", bufs=1)
                for ts in range(NTS):
                    nc.tensor.transpose(s_ps[:, ts * 128:(ts + 1) * 128],
                                        s_sb[:, ts * 128:(ts + 1) * 128],
                                        ident[:])
                stn_all = gpool.tile([128, T], f32, name="stn_all", tag="stn",
                                     bufs=2)
                # +1000 shift makes every score positive, so the chain can
                # mask out maxima with s*(s<r) in one fused op per round
                nc.scalar.activation(stn_all[:], s_ps[:], AF.Identity,
                                     bias=kshift[:], scale=1.0)
                return stn_all

            def emit_transpose_gates(ggs):
                g_ps = pp.tile([128, T], bf16, space="PSUM", name="g_ps",
                               tag="tr", bufs=1)
                for ts in range(NTS):
                    nc.tensor.transpose(g_ps[:, ts * 128:(ts + 1) * 128],
                                        ggs[ts][:], ident[:])
                gt_sb = gpool.tile([128, T], bf16, name="gt_sb", tag="gt_sb")
                nc.scalar.copy(gt_sb[:], g_ps[:])
                return gt_sb

            def emit_z_half(xa, q, half, zp):
                for k2 in range(NP // 2 * half, NP // 2 * (half + 1)):
                    nc.tensor.matmul(
                        zp[:],
                        az[q][:, k2 * 256:(k2 + 1) * 256]
                        .rearrange("p (two m) -> p two m", two=2),
                        xa[:, 2 * k2 * T:(2 * k2 + 2) * T]
                        .rearrange("p (two t) -> p two t", two=2),
                        start=(k2 == 0), stop=(k2 == NP - 1),
                        perf_mode=DR,
                    )

            def emit_gx_mul(st, q):
                gx = pp.tile([128, T], f32, space="PSUM", name="gx", tag="gx",
                             bufs=1)
                nc.tensor.matmul(gx[:],
                                 esb[:, q * 128:(q + 1) * 128],
                                 st["gt"][:],
                                 start=True, stop=True)
                nc.vector.tensor_mul(st["zbs"][q][:], st["zbs"][q][:], gx[:])

            def emit_delta_pairs(st, q, prs, veceng):
                # delta pair matmuls + PSUM->SBUF fp8 drains for pair
                # indices prs of quarter q of the previous macrotile
                for pr in prs:
                    hc = q * 8 + pr * 2
                    dp = pp.tile([128, 2 * T], f32, space="PSUM", name="dp",
                                 tag="dp", bufs=2)
                    for hf in range(2):
                        nc.tensor.matmul(
                            dp[:, hf * T:(hf + 1) * T],
                            bpk[:, (hc + hf) * 128:(hc + hf + 1) * 128],
                            st["zbs"][q][:],
                            start=True, stop=True)
                    if veceng == "both":
                        # tail drain: no later z-phase to hide behind, so
                        # both engines drain one half of every pair
                        nc.scalar.copy(st["da"][:, hc * T:(hc + 1) * T],
                                       dp[:, 0:T])
                        nc.vector.tensor_copy(
                            st["da"][:, (hc + 1) * T:(hc + 2) * T],
                            dp[:, T:2 * T])
                    elif pr in veceng:
                        nc.vector.tensor_copy(st["da"][:, hc * T:(hc + 2) * T],
                                              dp[:])
                    else:
                        nc.scalar.copy(st["da"][:, hc * T:(hc + 2) * T],
                                       dp[:])

            def emit_store(st, q):
                smt = st["mt"]
                nc.sync.dma_start(
                    yt_ap[:, (smt * 4 + q) * 8 * T:(smt * 4 + q + 1) * 8 * T],
                    st["da"][:, q * 8 * T:(q + 1) * 8 * T],
                )

            def emit_chain(stn_all):
                # the DVE gating chain for all 4 token sub-tiles
                ggs = []
                for ts in range(NTS):
                    stn = stn_all[:, ts * 128:(ts + 1) * 128]
                    r1 = spool.tile([128, 1], f32, name="r1", tag="r1")
                    nc.vector.reduce_max(r1[:], stn, axis=AX.X)
                    s2 = gpool.tile([128, 128], f32, name="s2", tag="s2")
                    nc.vector.scalar_tensor_tensor(s2[:], stn, r1[:], stn,
                                                   AL.is_lt, AL.mult)
                    r2 = spool.tile([128, 1], f32, name="r2", tag="r2")
                    nc.vector.reduce_max(r2[:], s2[:], axis=AX.X)
                    s3 = gpool.tile([128, 128], f32, name="s3", tag="s3")
                    nc.vector.scalar_tensor_tensor(s3[:], s2[:], r2[:], s2[:],
                                                   AL.is_lt, AL.mult)
                    r3 = spool.tile([128, 1], f32, name="r3", tag="r3")
                    nc.vector.reduce_max(r3[:], s3[:], axis=AX.X)
                    nr1 = spool.tile([128, 1], f32, name="nr1", tag="nr1")
                    nc.vector.tensor_scalar_mul(nr1[:], r1[:], -1.0)
                    ex = gpool.tile([128, 128], f32, name="ex", tag="ex")
                    nc.scalar.activation(ex[:], stn, AF.Exp, bias=nr1[:],
                                         scale=1.0)
                    em = gpool.tile([128, 128], f32, name="em", tag="em")
                    zs = spool.tile([128, 1], f32, name="zs", tag="zs")
                    nc.vector.scalar_tensor_tensor(em[:], stn, r3[:], ex[:],
                                                   AL.is_ge, AL.mult,
                                                   accum_out=zs[:])
                    rz = spool.tile([128, 1], f32, name="rz", tag="rz")
                    nc.vector.reciprocal(rz[:], zs[:])
                    gg = gpool.tile([128, 128], bf16, name="gg", tag="gg",
                                    bufs=NTS + 1)
                    nc.vector.tensor_scalar_mul(gg[:], em[:], rz[:])
                    ggs.append(gg)
                return ggs

            # ---------------- the pipelined macrotile loop ----------------
            prev = None  # state of the previous macrotile
            for mt in range(NMT):
                xa = xpool.tile([128, NKT * T], f8, name="xa", tag="xa")
                PART = NKT * T // 4
                for part in range(4):
                    nc.sync.dma_start(
                        xa[:, part * PART:(part + 1) * PART],
                        xt_ap[:, mt * NKT * T + part * PART:
                              mt * NKT * T + (part + 1) * PART])
                if mt == 0:
                    emit_const_loads()

                s_sb = emit_scores(xa)
                stn_all = emit_transpose_scores(s_sb)
                if prev is not None:
                    prev["gt"] = emit_transpose_gates(prev["ggs"])

                da = dpool.tile([128, NKT * T], f8, name="da", tag="da")
                cur = {"mt": mt, "da": da, "zbs": [], "ggs": None}

                for q in range(4):
                    if prev is not None:
                        emit_gx_mul(prev, q)
                        emit_delta_pairs(prev, q, (0, 1), veceng={1})
                    zp = pp.tile([128, T], f32, space="PSUM", name="zp",
                                 tag="zp")
                    emit_z_half(xa, q, 0, zp)
                    if prev is not None:
                        emit_delta_pairs(prev, q, (2, 3),
                                         veceng={3} if (q % 2 or mt == NMT - 1)
                                         else set())
                    emit_z_half(xa, q, 1, zp)
                    if prev is not None:
                        emit_store(prev, q)
                    zb = zpool.tile([128, T], bf16, name="zb", tag="zb")
                    nc.scalar.activation(zb[:], zp[:], AF.Identity,
                                         bias=bias[:, q:q + 1], scale=1.0)
                    cur["zbs"].append(zb)

                cur["ggs"] = emit_chain(stn_all)
                prev = cur

            # ---- drain the pipeline: delta of the last macrotile ----
            prev["gt"] = emit_transpose_gates(prev["ggs"])
            # tail drain: all 4 gate-expand matmuls + zg muls run first
            # (each mul frees the single gx bank; the vector queue is empty
            # so the round-trips are short), then the delta matmul stream
            # runs without any vector-queue interruptions.  The z-phase
            # "zp" PSUM tiles are dead now, so the delta chunks cycle
            # through dp pairs AND zp singles (6 chunks in flight).
            for q in range(4):
                emit_gx_mul(prev, q)
            for q in range(4):
                zgq = prev["zbs"][q]
                for grp in range(2):           # chunks [0..3] then [4..7]
                    hc0 = q * 8 + grp * 4
                    dp = pp.tile([128, 2 * T], f32, space="PSUM", name="dp",
                                 tag="dp", bufs=2)
                    for hf in range(2):
                        nc.tensor.matmul(
                            dp[:, hf * T:(hf + 1) * T],
                            bpk[:, (hc0 + hf) * 128:(hc0 + hf + 1) * 128],
                            zgq[:], start=True, stop=True)
                    nc.scalar.copy(prev["da"][:, hc0 * T:(hc0 + 1) * T],
                                   dp[:, 0:T])
                    nc.vector.tensor_copy(
                        prev["da"][:, (hc0 + 1) * T:(hc0 + 2) * T],
                        dp[:, T:2 * T])
                    for sng in range(2):
                        hc = hc0 + 2 + sng
                        zps = pp.tile([128, T], f32, space="PSUM", name="zp",
                                      tag="zp")
                        nc.tensor.matmul(zps[:],
                                         bpk[:, hc * 128:(hc + 1) * 128],
                                         zgq[:], start=True, stop=True)
                        dsl = prev["da"][:, hc * T:(hc + 1) * T]
                        if sng == 0:
                            nc.scalar.copy(dsl, zps[:])
                        else:
                            nc.vector.tensor_copy(dsl, zps[:])
                emit_store(prev, q)

    nc.compile()
    return nc


def _prep_consts(task_emb, task_ids, Wp, bp, centers, A, Bm, adapter_scale):
    scale = float(np.asarray(adapter_scale))
    A_all = np.ascontiguousarray(
        A.transpose(1, 0, 2).reshape(H, NB * R).astype(np.float32))
    W2 = (Wp @ centers.T).astype(np.float32)                     # [H, 128]

    # ah: [p, q, k2, two, m] = A_all[(2*k2+two)*128+p, q*128+m], fp8 e4m3
    # (DoubleRow pairs of consecutive k-tiles interleave along the free dim)
    ah = (A_all.reshape(NKT, 128, 4, 128).transpose(1, 2, 0, 3)
          .reshape(128, 4 * NKT * 128).astype(ml_dtypes.float8_e4m3))
    ah = np.ascontiguousarray(ah)
    # ws: [p, k2, two, m] = W2[(2*k2+two)*128+p, m], fp8 (DoubleRow pairs)
    wsn = np.ascontiguousarray(
        W2.reshape(NKT, 128, 128).transpose(1, 0, 2).reshape(128, NKT * 128)
        .astype(ml_dtypes.float8_e4m3))

    # block-diag up-projection, K=128 per h-chunk
    bpk = np.zeros((128, NKT * 128), np.float32)
    for hc in range(NKT):
        for mblk in range(4):
            n = hc * 4 + mblk
            for r in range(R):
                row = (hc % 8) * 16 + mblk * 4 + r
                bpk[row, hc * 128 + mblk * 32: hc * 128 + mblk * 32 + 32] = \
                    Bm[n, r, :] * scale
    bpk = bpk.astype(ml_dtypes.bfloat16)

    e_np = (np.arange(128)[:, None] == (np.arange(512)[None, :] // 4)) \
        .astype(ml_dtypes.bfloat16)

    sconst = (bp @ centers.T - 0.5 * (centers ** 2).sum(-1)).astype(np.float32)

    biases = []
    for c in range(NCORES):
        te = task_emb[int(np.asarray(task_ids)[c // 2])].astype(np.float32)
        b5 = np.empty((128, 5), np.float32)
        zoff = te @ A_all                                        # [512]
        for q in range(4):
            b5[:, q] = zoff[q * 128:(q + 1) * 128]
        b5[:, 4] = te @ W2 + sconst
        biases.append(np.ascontiguousarray(b5))
    return ah, wsn, bpk, e_np, biases


def kernel(x, task_ids, task_emb, Wp, bp, centers, A, Bm, adapter_scale):
    global _COMPILED, LAST_RESULT
    from concourse import bass_utils

    x = np.asarray(x, dtype=np.float32)
    task_ids = np.asarray(task_ids)
    task_emb = np.asarray(task_emb, dtype=np.float32)
    Wp = np.asarray(Wp, dtype=np.float32)
    bp = np.asarray(bp, dtype=np.float32)
    centers = np.asarray(centers, dtype=np.float32)
    A = np.asarray(A, dtype=np.float32)
    Bm = np.asarray(Bm, dtype=np.float32)

    if _COMPILED is None:
        _COMPILED = _build()
    nc = _COMPILED

    ah, wsn, bpk, e_np, biases = _prep_consts(
        task_emb, task_ids, Wp, bp, centers, A, Bm, adapter_scale)

    xf = x.reshape(B * S, H)
    xf8 = xf.astype(ml_dtypes.float8_e4m3)
    in_maps = []
    for c in range(NCORES):
        xtc = xf8[c * TPC:(c + 1) * TPC].reshape(NMT, T, NKT, 128)
        xtc = np.ascontiguousarray(xtc.transpose(3, 0, 2, 1)) \
            .reshape(128, NMT * NKT * T)
        in_maps.append({"xt": xtc, "ah": ah, "ws": wsn, "bpk": bpk,
                       "e": e_np, "bias": biases[c]})

    kwargs = {}
    if TRACE:
        kwargs = dict(trace=True, tmpdir=TRACE_DIR)
    res = bass_utils.run_bass_kernel_spmd(
        nc, in_maps, core_ids=list(range(NCORES)), **kwargs)
    LAST_RESULT = res

    out = np.empty((B * S, H), np.float32)
    for c in range(NCORES):
        dat = res.results[c]["yt"].reshape(128, NMT, 4, 8, T)
        delta = dat.transpose(1, 4, 2, 3, 0).reshape(TPC, H)
        out[c * TPC:(c + 1) * TPC] = xf[c * TPC:(c + 1) * TPC] + \
            delta.astype(np.float32)
    return out.reshape(B, S, H)


# revision 21
# speedup vs baseline: 1.1412x; 1.0089x over previous
"""Trainium2 Bass kernel for NeuroplasticLlama block-sparse adapter (moe_routing).

Contract: kernel(**inputs) takes FULL unsharded inputs (as produced by
setup_inputs) and returns the FULL [4, 4096, 4096] float32 output.

Strategy (data/sequence parallel over 8 cores, 2048 tokens each):
  - Each core's 2048 contiguous tokens belong to exactly one batch, so the
    task embedding contributes only per-core constant bias vectors
    (te @ A, te @ W2) -- h = x + te is never materialized.
  - Dense formulation of the routed computation:
      scores s[t,n] = x @ (Wp @ centers.T)[:,n] + const_n (shift dropped;
        top-k set and softmax are shift invariant)
      top-3 via threshold = 3rd max (3 rounds of max + mask-out)
      gates g[t,n] = exp(s - max) * (s >= thr3) / sum(...)
      z[t,:] = x @ A_all (all 512 block-rank pairs, dense)
      zg = z * expand4(g);  delta = block-diag(Bm) matmul
  - The device computes only DELTA (fp8 in, fp8 out); the residual
    y = x + delta is applied on the host during unsharding, keeping the
    x term exact and HBM traffic at 2 MB in + 2 MB out per 512-token
    macrotile per core.
  - x is fed pre-transposed and pre-shuffled to the [p][mt][k][t] layout
    so every DMA is a contiguous [128, N] transfer (128 descriptors; a
    strided [H, tokens] slice costs ~18us of descriptor generation).
  - scores and z are fp8 DoubleRow matmuls off the same x tile; delta is
    bf16 (output-stream bound, DoubleRow would not help).
  - SOFTWARE PIPELINE: macrotile m's delta/gate-apply phase executes
    during macrotile m+1's scores/z phase, with delta pair-matmuls
    interleaved between z chunks.  The PSUM->SBUF delta drains (the
    throughput-critical ~9us/mt of scalar+vector work) then overlap the
    z matmuls instead of bursting at the end of each macrotile, and the
    gating chain of m+1 runs in the vector queue after m's drains.
"""

import sys

if "/opt/trn_rl_repo" not in sys.path:
    sys.path.insert(0, "/opt/trn_rl_repo")

import numpy as np
import ml_dtypes

H = 4096
NB = 128
BLK = 32
R = 4
B = 4
S = 4096
NCORES = 8
TPC = (B * S) // NCORES  # tokens per core = 2048
T = 512                  # tokens per macrotile
NMT = TPC // T           # 4 macrotiles per core
NKT = H // 128           # 32 k-tiles over the hidden dim
NP = NKT // 2            # 16 DoubleRow k-pair tiles
BIG = 1.0e30

TRACE = False            # set by test.py for profiling runs
TRACE_DIR = None
LAST_RESULT = None       # BassKernelResults of the last run

_COMPILED = None


def _build():
    import concourse.bacc as bacc
    import concourse.tile as tile
    from concourse import mybir, masks

    f32 = mybir.dt.float32
    bf16 = mybir.dt.bfloat16
    f8 = mybir.dt.float8e4
    AF = mybir.ActivationFunctionType
    AL = mybir.AluOpType
    AX = mybir.AxisListType
    DR = mybir.MatmulPerfMode.DoubleRow

    nc = bacc.Bacc("TRN2", target_bir_lowering=False, debug=False,
                   num_devices=NCORES)

    xt_d = nc.dram_tensor("xt", [128, NMT * NKT * T], f8, kind="ExternalInput")
    ah_d = nc.dram_tensor("ah", [128, 4 * NKT * 128], f8, kind="ExternalInput")
    ws_d = nc.dram_tensor("ws", [128, NKT * 128], f8, kind="ExternalInput")
    bpk_d = nc.dram_tensor("bpk", [128, NKT * 128], bf16, kind="ExternalInput")
    e_d = nc.dram_tensor("e", [128, 512], bf16, kind="ExternalInput")
    bias_d = nc.dram_tensor("bias", [128, 5], f32, kind="ExternalInput")
    yt_d = nc.dram_tensor("yt", [128, NMT * NKT * T], f8, kind="ExternalOutput")

    xt_ap = xt_d.ap()
    yt_ap = yt_d.ap()

    with tile.TileContext(nc) as tc:
        from contextlib import ExitStack
        with ExitStack() as ctx:
            cpool = ctx.enter_context(tc.tile_pool(name="consts", bufs=1))
            xpool = ctx.enter_context(tc.tile_pool(name="xg", bufs=2))
            dpool = ctx.enter_context(tc.tile_pool(name="dall", bufs=2))
            zpool = ctx.enter_context(tc.tile_pool(name="zb", bufs=8))
            gpool = ctx.enter_context(tc.tile_pool(name="gate", bufs=3))
            spool = ctx.enter_context(tc.tile_pool(name="scal", bufs=4))
            pp = ctx.enter_context(tc.tile_pool(name="ps", bufs=2, space="PSUM"))

            # ---- persistent constants ----
            # xa0 is issued FIRST (in the macrotile loop below) on the sync
            # HWDGE ring; az/bpk/esb queue BEHIND it on the same ring so the
            # first scores matmul's input gets the full HBM bandwidth
            # (per-queue FIFO), while the tiny ws/bias go in parallel on
            # scalar/gpsimd.  az0 is only needed ~4us after scores start.
            ws = cpool.tile([128, NKT * 128], f8, name="ws", tag="ws")
            nc.scalar.dma_start(ws[:, 0:8 * 256], ws_d.ap()[:, 0:8 * 256])
            nc.scalar.dma_start(ws[:, 8 * 256:], ws_d.ap()[:, 8 * 256:])
            bias = cpool.tile([128, 5], f32, name="bias", tag="bias")
            nc.gpsimd.dma_start(bias[:], bias_d.ap()[:])
            ident = cpool.tile([128, 128], bf16, name="ident", tag="ident")
            masks.make_identity(nc, ident[:])
            kshift = cpool.tile([128, 1], f32, name="kshift", tag="kshift")
            nc.gpsimd.memset(kshift[:], 1000.0)
            az = []
            bpk = cpool.tile([128, NKT * 128], bf16, name="bpk", tag="bpk")
            esb = cpool.tile([128, 512], bf16, name="esb", tag="esb")

            def emit_const_loads():
                for q in range(4):
                    t_az = cpool.tile([128, NKT * 128], f8, name=f"az{q}",
                                      tag=f"az{q}")
                    nc.sync.dma_start(
                        t_az[:],
                        ah_d.ap()[:, q * NKT * 128:(q + 1) * NKT * 128])
                    az.append(t_az)
                nc.sync.dma_start(bpk[:], bpk_d.ap()[:])
                nc.sync.dma_start(esb[:], e_d.ap()[:])

            NTS = T // 128  # token sub-tiles per macrotile

            # ---------------- pipelined stage helpers ----------------
            def emit_scores(xa):
                sp = pp.tile([128, T], f32, space="PSUM", name="sp", tag="zp")
                for k2 in range(NP):
                    nc.tensor.matmul(
                        sp[:],
                        ws[:, k2 * 256:(k2 + 1) * 256]
                        .rearrange("p (two m) -> p two m", two=2),
                        xa[:, 2 * k2 * T:(2 * k2 + 2) * T]
                        .rearrange("p (two t) -> p two t", two=2),
                        start=(k2 == 0), stop=(k2 == NP - 1),
                        perf_mode=DR,
                    )
                s_sb = gpool.tile([128, T], bf16, name="s_sb", tag="s_sb")
                nc.scalar.activation(s_sb[:], sp[:], AF.Identity,
                                     bias=bias[:, 4:5], scale=1.0)
                return s_sb

            def emit_transpose_scores(s_sb):
                s_ps = pp.tile([128, T], bf16, space="PSUM", name="s_ps",
                               tag="tr", bufs=1)
                for ts in range(NTS):
                    nc.tensor.transpose(s_ps[:, ts * 128:(ts + 1) * 128],
                                        s_sb[:, ts * 128:(ts + 1) * 128],
                                        ident[:])
                stn_all = gpool.tile([128, T], f32, name="stn_all", tag="stn",
                                     bufs=2)
                # +1000 shift makes every score positive, so the chain can
                # mask out maxima with s*(s<r) in one fused op per round
                nc.scalar.activation(stn_all[:], s_ps[:], AF.Identity,
                                     bias=kshift[:], scale=1.0)
                return stn_all

            def emit_transpose_gates(ggs):
                g_ps = pp.tile([128, T], bf16, space="PSUM", name="g_ps",
                               tag="tr", bufs=1)
                for ts in range(NTS):
                    nc.tensor.transpose(g_ps[:, ts * 128:(ts + 1) * 128],
                                        ggs[ts][:], ident[:])
                gt_sb = gpool.tile([128, T], bf16, name="gt_sb", tag="gt_sb")
                nc.scalar.copy(gt_sb[:], g_ps[:])
                return gt_sb

            def emit_z_half(xa, q, half, zp):
                for k2 in range(NP // 2 * half, NP // 2 * (half + 1)):
                    nc.tensor.matmul(
                        zp[:],
                        az[q][:, k2 * 256:(k2 + 1) * 256]
                        .rearrange("p (two m) -> p two m", two=2),
                        xa[:, 2 * k2 * T:(2 * k2 + 2) * T]
                        .rearrange("p (two t) -> p two t", two=2),
                        start=(k2 == 0), stop=(k2 == NP - 1),
                        perf_mode=DR,
                    )

            def emit_gx_mul(st, q):
                gx = pp.tile([128, T], f32, space="PSUM", name="gx", tag="gx",
                             bufs=1)
                nc.tensor.matmul(gx[:],
                                 esb[:, q * 128:(q + 1) * 128],
                                 st["gt"][:],
                                 start=True, stop=True)
                nc.vector.tensor_mul(st["zbs"][q][:], st["zbs"][q][:], gx[:])

            def emit_delta_pairs(st, q, prs, veceng):
                # delta pair matmuls + PSUM->SBUF fp8 drains for pair
                # indices prs of quarter q of the previous macrotile
                for pr in prs:
                    hc = q * 8 + pr * 2
                    dp = pp.tile([128, 2 * T], f32, space="PSUM", name="dp",
                                 tag="dp", bufs=2)
                    for hf in range(2):
                        nc.tensor.matmul(
                            dp[:, hf * T:(hf + 1) * T],
                            bpk[:, (hc + hf) * 128:(hc + hf + 1) * 128],
                            st["zbs"][q][:],
                            start=True, stop=True)
                    if veceng == "both":
                        # tail drain: no later z-phase to hide behind, so
                        # both engines drain one half of every pair
                        nc.scalar.copy(st["da"][:, hc * T:(hc + 1) * T],
                                       dp[:, 0:T])
                        nc.vector.tensor_copy(
                            st["da"][:, (hc + 1) * T:(hc + 2) * T],
                            dp[:, T:2 * T])
                    elif pr in veceng:
                        nc.vector.tensor_copy(st["da"][:, hc * T:(hc + 2) * T],
                                              dp[:])
                    else:
                        nc.scalar.copy(st["da"][:, hc * T:(hc + 2) * T],
                                       dp[:])

            def emit_store(st, q):
                smt = st["mt"]
                nc.sync.dma_start(
                    yt_ap[:, (smt * 4 + q) * 8 * T:(smt * 4 + q + 1) * 8 * T],
                    st["da"][:, q * 8 * T:(q + 1) * 8 * T],
                )

            def emit_chain(stn_all, only_ts=None):
                # the DVE gating chain for token sub-tiles
                ggs = []
                tslist = range(NTS) if only_ts is None else [only_ts]
                for ts in tslist:
                    stn = stn_all[:, ts * 128:(ts + 1) * 128]
                    r1 = spool.tile([128, 1], f32, name="r1", tag="r1")
                    nc.vector.reduce_max(r1[:], stn, axis=AX.X)
                    s2 = gpool.tile([128, 128], f32, name="s2", tag="s2")
                    nc.vector.scalar_tensor_tensor(s2[:], stn, r1[:], stn,
                                                   AL.is_lt, AL.mult)
                    r2 = spool.tile([128, 1], f32, name="r2", tag="r2")
                    nc.vector.reduce_max(r2[:], s2[:], axis=AX.X)
                    s3 = gpool.tile([128, 128], f32, name="s3", tag="s3")
                    nc.vector.scalar_tensor_tensor(s3[:], s2[:], r2[:], s2[:],
                                                   AL.is_lt, AL.mult)
                    r3 = spool.tile([128, 1], f32, name="r3", tag="r3")
                    nc.vector.reduce_max(r3[:], s3[:], axis=AX.X)
                    nr1 = spool.tile([128, 1], f32, name="nr1", tag="nr1")
                    nc.vector.tensor_scalar_mul(nr1[:], r1[:], -1.0)
                    ex = gpool.tile([128, 128], f32, name="ex", tag="ex")
                    nc.scalar.activation(ex[:], stn, AF.Exp, bias=nr1[:],
                                         scale=1.0)
                    em = gpool.tile([128, 128], f32, name="em", tag="em")
                    zs = spool.tile([128, 1], f32, name="zs", tag="zs")
                    nc.vector.scalar_tensor_tensor(em[:], stn, r3[:], ex[:],
                                                   AL.is_ge, AL.mult,
                                                   accum_out=zs[:])
                    rz = spool.tile([128, 1], f32, name="rz", tag="rz")
                    nc.vector.reciprocal(rz[:], zs[:])
                    gg = gpool.tile([128, 128], bf16, name="gg", tag="gg",
                                    bufs=NTS + 1)
                    nc.vector.tensor_scalar_mul(gg[:], em[:], rz[:])
                    ggs.append(gg)
                return ggs

            def emit_chain_ts(stn_all, ts):
                return emit_chain(stn_all, only_ts=ts)[0]

            # ---------------- the pipelined macrotile loop ----------------
            prev = None  # state of the previous macrotile
            lastggs = []
            for mt in range(NMT):
                xa = xpool.tile([128, NKT * T], f8, name="xa", tag="xa")
                PART = NKT * T // 4
                for part in range(4):
                    nc.sync.dma_start(
                        xa[:, part * PART:(part + 1) * PART],
                        xt_ap[:, mt * NKT * T + part * PART:
                              mt * NKT * T + (part + 1) * PART])
                if mt == 0:
                    emit_const_loads()

                s_sb = emit_scores(xa)
                stn_all = emit_transpose_scores(s_sb)
                if prev is not None:
                    prev["gt"] = emit_transpose_gates(prev["ggs"])

                da = dpool.tile([128, NKT * T], f8, name="da", tag="da")
                cur = {"mt": mt, "da": da, "zbs": [], "ggs": None}

                for q in range(4):
                    if prev is not None:
                        emit_gx_mul(prev, q)
                        emit_delta_pairs(prev, q, (0, 1), veceng={1})
                    zp = pp.tile([128, T], f32, space="PSUM", name="zp",
                                 tag="zp")
                    emit_z_half(xa, q, 0, zp)
                    if prev is not None:
                        emit_delta_pairs(prev, q, (2, 3),
                                         veceng={3} if (q % 2 or mt == NMT - 1)
                                         else set())
                    emit_z_half(xa, q, 1, zp)
                    if prev is not None:
                        emit_store(prev, q)
                    zb = zpool.tile([128, T], bf16, name="zb", tag="zb")
                    nc.scalar.activation(zb[:], zp[:], AF.Identity,
                                         bias=bias[:, q:q + 1], scale=1.0)
                    cur["zbs"].append(zb)
                    if mt == NMT - 1:
                        # last macrotile: its chain competes with the tail,
                        # so spread the sub-chains across the z quarters
                        lastggs.append(emit_chain_ts(stn_all, q))

                if mt == NMT - 1:
                    cur["ggs"] = lastggs
                else:
                    cur["ggs"] = emit_chain(stn_all)
                prev = cur

            # ---- drain the pipeline: delta of the last macrotile ----
            prev["gt"] = emit_transpose_gates(prev["ggs"])
            # tail drain: all 4 gate-expand matmuls + zg muls run first
            # (each mul frees the single gx bank; the vector queue is empty
            # so the round-trips are short), then the delta matmul stream
            # runs without any vector-queue interruptions.  The z-phase
            # "zp" PSUM tiles are dead now, so the delta chunks cycle
            # through dp pairs AND zp singles (6 chunks in flight).
            for q in range(4):
                emit_gx_mul(prev, q)
            for q in range(4):
                zgq = prev["zbs"][q]
                for grp in range(2):           # chunks [0..3] then [4..7]
                    hc0 = q * 8 + grp * 4
                    dp = pp.tile([128, 2 * T], f32, space="PSUM", name="dp",
                                 tag="dp", bufs=2)
                    for hf in range(2):
                        nc.tensor.matmul(
                            dp[:, hf * T:(hf + 1) * T],
                            bpk[:, (hc0 + hf) * 128:(hc0 + hf + 1) * 128],
                            zgq[:], start=True, stop=True)
                    nc.scalar.copy(prev["da"][:, hc0 * T:(hc0 + 1) * T],
                                   dp[:, 0:T])
                    nc.vector.tensor_copy(
                        prev["da"][:, (hc0 + 1) * T:(hc0 + 2) * T],
                        dp[:, T:2 * T])
                    for sng in range(2):
                        hc = hc0 + 2 + sng
                        zps = pp.tile([128, T], f32, space="PSUM", name="zp",
                                      tag="zp")
                        nc.tensor.matmul(zps[:],
                                         bpk[:, hc * 128:(hc + 1) * 128],
                                         zgq[:], start=True, stop=True)
                        dsl = prev["da"][:, hc * T:(hc + 1) * T]
                        if sng == 0:
                            nc.scalar.copy(dsl, zps[:])
                        else:
                            nc.vector.tensor_copy(dsl, zps[:])
                emit_store(prev, q)

    nc.compile()
    return nc


def _prep_consts(task_emb, task_ids, Wp, bp, centers, A, Bm, adapter_scale):
    scale = float(np.asarray(adapter_scale))
    A_all = np.ascontiguousarray(
        A.transpose(1, 0, 2).reshape(H, NB * R).astype(np.float32))
    W2 = (Wp @ centers.T).astype(np.float32)                     # [H, 128]

    # ah: [p, q, k2, two, m] = A_all[(2*k2+two)*128+p, q*128+m], fp8 e4m3
    # (DoubleRow pairs of consecutive k-tiles interleave along the free dim)
    ah = (A_all.reshape(NKT, 128, 4, 128).transpose(1, 2, 0, 3)
          .reshape(128, 4 * NKT * 128).astype(ml_dtypes.float8_e4m3))
    ah = np.ascontiguousarray(ah)
    # ws: [p, k2, two, m] = W2[(2*k2+two)*128+p, m], fp8 (DoubleRow pairs)
    wsn = np.ascontiguousarray(
        W2.reshape(NKT, 128, 128).transpose(1, 0, 2).reshape(128, NKT * 128)
        .astype(ml_dtypes.float8_e4m3))

    # block-diag up-projection, K=128 per h-chunk
    bpk = np.zeros((128, NKT * 128), np.float32)
    for hc in range(NKT):
        for mblk in range(4):
            n = hc * 4 + mblk
            for r in range(R):
                row = (hc % 8) * 16 + mblk * 4 + r
                bpk[row, hc * 128 + mblk * 32: hc * 128 + mblk * 32 + 32] = \
                    Bm[n, r, :] * scale
    bpk = bpk.astype(ml_dtypes.bfloat16)

    e_np = (np.arange(128)[:, None] == (np.arange(512)[None, :] // 4)) \
        .astype(ml_dtypes.bfloat16)

    sconst = (bp @ centers.T - 0.5 * (centers ** 2).sum(-1)).astype(np.float32)

    biases = []
    for c in range(NCORES):
        te = task_emb[int(np.asarray(task_ids)[c // 2])].astype(np.float32)
        b5 = np.empty((128, 5), np.float32)
        zoff = te @ A_all                                        # [512]
        for q in range(4):
            b5[:, q] = zoff[q * 128:(q + 1) * 128]
        b5[:, 4] = te @ W2 + sconst
        biases.append(np.ascontiguousarray(b5))
    return ah, wsn, bpk, e_np, biases


def kernel(x, task_ids, task_emb, Wp, bp, centers, A, Bm, adapter_scale):
    global _COMPILED, LAST_RESULT
    from concourse import bass_utils

    x = np.asarray(x, dtype=np.float32)
    task_ids = np.asarray(task_ids)
    task_emb = np.asarray(task_emb, dtype=np.float32)
    Wp = np.asarray(Wp, dtype=np.float32)
    bp = np.asarray(bp, dtype=np.float32)
    centers = np.asarray(centers, dtype=np.float32)
    A = np.asarray(A, dtype=np.float32)
    Bm = np.asarray(Bm, dtype=np.float32)

    if _COMPILED is None:
        _COMPILED = _build()
    nc = _COMPILED

    ah, wsn, bpk, e_np, biases = _prep_consts(
        task_emb, task_ids, Wp, bp, centers, A, Bm, adapter_scale)

    xf = x.reshape(B * S, H)
    xf8 = xf.astype(ml_dtypes.float8_e4m3)
    in_maps = []
    for c in range(NCORES):
        xtc = xf8[c * TPC:(c + 1) * TPC].reshape(NMT, T, NKT, 128)
        xtc = np.ascontiguousarray(xtc.transpose(3, 0, 2, 1)) \
            .reshape(128, NMT * NKT * T)
        in_maps.append({"xt": xtc, "ah": ah, "ws": wsn, "bpk": bpk,
                       "e": e_np, "bias": biases[c]})

    kwargs = {}
    if TRACE:
        kwargs = dict(trace=True, tmpdir=TRACE_DIR)
    res = bass_utils.run_bass_kernel_spmd(
        nc, in_maps, core_ids=list(range(NCORES)), **kwargs)
    LAST_RESULT = res

    out = np.empty((B * S, H), np.float32)
    for c in range(NCORES):
        dat = res.results[c]["yt"].reshape(128, NMT, 4, 8, T)
        delta = dat.transpose(1, 4, 2, 3, 0).reshape(TPC, H)
        out[c * TPC:(c + 1) * TPC] = xf[c * TPC:(c + 1) * TPC] + \
            delta.astype(np.float32)
    return out.reshape(B, S, H)


# revision 22
# speedup vs baseline: 1.1550x; 1.0121x over previous
"""Trainium2 Bass kernel for NeuroplasticLlama block-sparse adapter (moe_routing).

Contract: kernel(**inputs) takes FULL unsharded inputs (as produced by
setup_inputs) and returns the FULL [4, 4096, 4096] float32 output.

Strategy (data/sequence parallel over 8 cores, 2048 tokens each):
  - Each core's 2048 contiguous tokens belong to exactly one batch, so the
    task embedding contributes only per-core constant bias vectors
    (te @ A, te @ W2) -- h = x + te is never materialized.
  - Dense formulation of the routed computation:
      scores s[t,n] = x @ (Wp @ centers.T)[:,n] + const_n (shift dropped;
        top-k set and softmax are shift invariant)
      top-3 via threshold = 3rd max (3 rounds of max + mask-out)
      gates g[t,n] = exp(s - max) * (s >= thr3) / sum(...)
      z[t,:] = x @ A_all (all 512 block-rank pairs, dense)
      zg = z * expand4(g);  delta = block-diag(Bm) matmul
  - The device computes only DELTA (fp8 in, fp8 out); the residual
    y = x + delta is applied on the host during unsharding, keeping the
    x term exact and HBM traffic at 2 MB in + 2 MB out per 512-token
    macrotile per core.
  - x is fed pre-transposed and pre-shuffled to the [p][mt][k][t] layout
    so every DMA is a contiguous [128, N] transfer (128 descriptors; a
    strided [H, tokens] slice costs ~18us of descriptor generation).
  - scores and z are fp8 DoubleRow matmuls off the same x tile; delta is
    bf16 (output-stream bound, DoubleRow would not help).
  - SOFTWARE PIPELINE: macrotile m's delta/gate-apply phase executes
    during macrotile m+1's scores/z phase, with delta pair-matmuls
    interleaved between z chunks.  The PSUM->SBUF delta drains (the
    throughput-critical ~9us/mt of scalar+vector work) then overlap the
    z matmuls instead of bursting at the end of each macrotile, and the
    gating chain of m+1 runs in the vector queue after m's drains.
"""

import sys

if "/opt/trn_rl_repo" not in sys.path:
    sys.path.insert(0, "/opt/trn_rl_repo")

import numpy as np
import ml_dtypes

H = 4096
NB = 128
BLK = 32
R = 4
B = 4
S = 4096
NCORES = 8
TPC = (B * S) // NCORES  # tokens per core = 2048
T = 512                  # tokens per macrotile
NMT = TPC // T           # 4 macrotiles per core
NKT = H // 128           # 32 k-tiles over the hidden dim
NP = NKT // 2            # 16 DoubleRow k-pair tiles
BIG = 1.0e30

TRACE = False            # set by test.py for profiling runs
TRACE_DIR = None
LAST_RESULT = None       # BassKernelResults of the last run

_COMPILED = None


def _build():
    import concourse.bacc as bacc
    import concourse.tile as tile
    from concourse import mybir, masks

    f32 = mybir.dt.float32
    bf16 = mybir.dt.bfloat16
    f8 = mybir.dt.float8e4
    AF = mybir.ActivationFunctionType
    AL = mybir.AluOpType
    AX = mybir.AxisListType
    DR = mybir.MatmulPerfMode.DoubleRow

    nc = bacc.Bacc("TRN2", target_bir_lowering=False, debug=False,
                   num_devices=NCORES)

    xt_d = nc.dram_tensor("xt", [128, NMT * NKT * T], f8, kind="ExternalInput")
    ah_d = nc.dram_tensor("ah", [128, 4 * NKT * 128], f8, kind="ExternalInput")
    ws_d = nc.dram_tensor("ws", [128, NKT * 128], f8, kind="ExternalInput")
    bpk_d = nc.dram_tensor("bpk", [128, NKT * 128], bf16, kind="ExternalInput")
    e_d = nc.dram_tensor("e", [128, 512], bf16, kind="ExternalInput")
    bias_d = nc.dram_tensor("bias", [128, 5], f32, kind="ExternalInput")
    yt_d = nc.dram_tensor("yt", [128, NMT * NKT * T], f8, kind="ExternalOutput")

    xt_ap = xt_d.ap()
    yt_ap = yt_d.ap()

    with tile.TileContext(nc) as tc:
        from contextlib import ExitStack
        with ExitStack() as ctx:
            cpool = ctx.enter_context(tc.tile_pool(name="consts", bufs=1))
            xpool = ctx.enter_context(tc.tile_pool(name="xg", bufs=2))
            dpool = ctx.enter_context(tc.tile_pool(name="dall", bufs=2))
            zpool = ctx.enter_context(tc.tile_pool(name="zb", bufs=8))
            gpool = ctx.enter_context(tc.tile_pool(name="gate", bufs=3))
            spool = ctx.enter_context(tc.tile_pool(name="scal", bufs=4))
            pp = ctx.enter_context(tc.tile_pool(name="ps", bufs=2, space="PSUM"))

            # ---- persistent constants ----
            # xa0 is issued FIRST (in the macrotile loop below) on the sync
            # HWDGE ring; az/bpk/esb queue BEHIND it on the same ring so the
            # first scores matmul's input gets the full HBM bandwidth
            # (per-queue FIFO), while the tiny ws/bias go in parallel on
            # scalar/gpsimd.  az0 is only needed ~4us after scores start.
            ws = cpool.tile([128, NKT * 128], f8, name="ws", tag="ws")
            nc.scalar.dma_start(ws[:, 0:8 * 256], ws_d.ap()[:, 0:8 * 256])
            nc.scalar.dma_start(ws[:, 8 * 256:], ws_d.ap()[:, 8 * 256:])
            bias = cpool.tile([128, 5], f32, name="bias", tag="bias")
            nc.gpsimd.dma_start(bias[:], bias_d.ap()[:])
            ident = cpool.tile([128, 128], bf16, name="ident", tag="ident")
            masks.make_identity(nc, ident[:])
            kshift = cpool.tile([128, 1], f32, name="kshift", tag="kshift")
            nc.gpsimd.memset(kshift[:], 1000.0)
            az = []
            bpk = cpool.tile([128, NKT * 128], bf16, name="bpk", tag="bpk")
            esb = cpool.tile([128, 512], bf16, name="esb", tag="esb")

            def emit_const_loads():
                for q in range(4):
                    t_az = cpool.tile([128, NKT * 128], f8, name=f"az{q}",
                                      tag=f"az{q}")
                    nc.sync.dma_start(
                        t_az[:],
                        ah_d.ap()[:, q * NKT * 128:(q + 1) * NKT * 128])
                    az.append(t_az)
                nc.sync.dma_start(bpk[:], bpk_d.ap()[:])
                nc.sync.dma_start(esb[:], e_d.ap()[:])

            NTS = T // 128  # token sub-tiles per macrotile

            # ---------------- pipelined stage helpers ----------------
            def emit_scores(xa):
                sp = pp.tile([128, T], f32, space="PSUM", name="sp", tag="zp")
                for k2 in range(NP):
                    nc.tensor.matmul(
                        sp[:],
                        ws[:, k2 * 256:(k2 + 1) * 256]
                        .rearrange("p (two m) -> p two m", two=2),
                        xa[:, 2 * k2 * T:(2 * k2 + 2) * T]
                        .rearrange("p (two t) -> p two t", two=2),
                        start=(k2 == 0), stop=(k2 == NP - 1),
                        perf_mode=DR,
                    )
                s_sb = gpool.tile([128, T], bf16, name="s_sb", tag="s_sb")
                nc.scalar.activation(s_sb[:], sp[:], AF.Identity,
                                     bias=bias[:, 4:5], scale=1.0)
                return s_sb

            def emit_transpose_scores(s_sb):
                s_ps = pp.tile([128, T], bf16, space="PSUM", name="s_ps",
                               tag="tr", bufs=1)
                for ts in range(NTS):
                    nc.tensor.transpose(s_ps[:, ts * 128:(ts + 1) * 128],
                                        s_sb[:, ts * 128:(ts + 1) * 128],
                                        ident[:])
                stn_all = gpool.tile([128, T], f32, name="stn_all", tag="stn",
                                     bufs=2)
                # +1000 shift makes every score positive, so the chain can
                # mask out maxima with s*(s<r) in one fused op per round
                nc.scalar.activation(stn_all[:], s_ps[:], AF.Identity,
                                     bias=kshift[:], scale=1.0)
                return stn_all

            def emit_transpose_gates(ggs):
                g_ps = pp.tile([128, T], bf16, space="PSUM", name="g_ps",
                               tag="tr", bufs=1)
                for ts in range(NTS):
                    nc.tensor.transpose(g_ps[:, ts * 128:(ts + 1) * 128],
                                        ggs[ts][:], ident[:])
                gt_sb = gpool.tile([128, T], bf16, name="gt_sb", tag="gt_sb")
                nc.scalar.copy(gt_sb[:], g_ps[:])
                return gt_sb

            def emit_z_part(xa, q, part, zp):
                # one quarter of the k2 range (4 DoubleRow matmuls)
                for k2 in range(NP // 4 * part, NP // 4 * (part + 1)):
                    nc.tensor.matmul(
                        zp[:],
                        az[q][:, k2 * 256:(k2 + 1) * 256]
                        .rearrange("p (two m) -> p two m", two=2),
                        xa[:, 2 * k2 * T:(2 * k2 + 2) * T]
                        .rearrange("p (two t) -> p two t", two=2),
                        start=(k2 == 0), stop=(k2 == NP - 1),
                        perf_mode=DR,
                    )

            def emit_gx_mul(st, q):
                gx = pp.tile([128, T], f32, space="PSUM", name="gx", tag="gx",
                             bufs=1)
                nc.tensor.matmul(gx[:],
                                 esb[:, q * 128:(q + 1) * 128],
                                 st["gt"][:],
                                 start=True, stop=True)
                nc.vector.tensor_mul(st["zbs"][q][:], st["zbs"][q][:], gx[:])

            def emit_delta_pairs(st, q, prs, veceng):
                # delta pair matmuls + PSUM->SBUF fp8 drains for pair
                # indices prs of quarter q of the previous macrotile
                for pr in prs:
                    hc = q * 8 + pr * 2
                    dp = pp.tile([128, 2 * T], f32, space="PSUM", name="dp",
                                 tag="dp", bufs=2)
                    for hf in range(2):
                        nc.tensor.matmul(
                            dp[:, hf * T:(hf + 1) * T],
                            bpk[:, (hc + hf) * 128:(hc + hf + 1) * 128],
                            st["zbs"][q][:],
                            start=True, stop=True)
                    if veceng == "both":
                        # tail drain: no later z-phase to hide behind, so
                        # both engines drain one half of every pair
                        nc.scalar.copy(st["da"][:, hc * T:(hc + 1) * T],
                                       dp[:, 0:T])
                        nc.vector.tensor_copy(
                            st["da"][:, (hc + 1) * T:(hc + 2) * T],
                            dp[:, T:2 * T])
                    elif pr in veceng:
                        nc.vector.tensor_copy(st["da"][:, hc * T:(hc + 2) * T],
                                              dp[:])
                    else:
                        nc.scalar.copy(st["da"][:, hc * T:(hc + 2) * T],
                                       dp[:])

            def emit_store(st, q):
                smt = st["mt"]
                nc.sync.dma_start(
                    yt_ap[:, (smt * 4 + q) * 8 * T:(smt * 4 + q + 1) * 8 * T],
                    st["da"][:, q * 8 * T:(q + 1) * 8 * T],
                )

            def emit_chain(stn_all, only_ts=None):
                # the DVE gating chain for token sub-tiles
                ggs = []
                tslist = range(NTS) if only_ts is None else [only_ts]
                for ts in tslist:
                    stn = stn_all[:, ts * 128:(ts + 1) * 128]
                    r1 = spool.tile([128, 1], f32, name="r1", tag="r1")
                    nc.vector.reduce_max(r1[:], stn, axis=AX.X)
                    s2 = gpool.tile([128, 128], f32, name="s2", tag="s2")
                    nc.vector.scalar_tensor_tensor(s2[:], stn, r1[:], stn,
                                                   AL.is_lt, AL.mult)
                    r2 = spool.tile([128, 1], f32, name="r2", tag="r2")
                    nc.vector.reduce_max(r2[:], s2[:], axis=AX.X)
                    s3 = gpool.tile([128, 128], f32, name="s3", tag="s3")
                    nc.vector.scalar_tensor_tensor(s3[:], s2[:], r2[:], s2[:],
                                                   AL.is_lt, AL.mult)
                    r3 = spool.tile([128, 1], f32, name="r3", tag="r3")
                    nc.vector.reduce_max(r3[:], s3[:], axis=AX.X)
                    nr1 = spool.tile([128, 1], f32, name="nr1", tag="nr1")
                    nc.vector.tensor_scalar_mul(nr1[:], r1[:], -1.0)
                    ex = gpool.tile([128, 128], f32, name="ex", tag="ex")
                    nc.scalar.activation(ex[:], stn, AF.Exp, bias=nr1[:],
                                         scale=1.0)
                    em = gpool.tile([128, 128], f32, name="em", tag="em")
                    zs = spool.tile([128, 1], f32, name="zs", tag="zs")
                    nc.vector.scalar_tensor_tensor(em[:], stn, r3[:], ex[:],
                                                   AL.is_ge, AL.mult,
                                                   accum_out=zs[:])
                    rz = spool.tile([128, 1], f32, name="rz", tag="rz")
                    nc.vector.reciprocal(rz[:], zs[:])
                    gg = gpool.tile([128, 128], bf16, name="gg", tag="gg",
                                    bufs=NTS + 1)
                    nc.vector.tensor_scalar_mul(gg[:], em[:], rz[:])
                    ggs.append(gg)
                return ggs

            def emit_chain_ts(stn_all, ts):
                return emit_chain(stn_all, only_ts=ts)[0]

            # ---------------- the pipelined macrotile loop ----------------
            prev = None  # state of the previous macrotile
            lastggs = []
            for mt in range(NMT):
                xa = xpool.tile([128, NKT * T], f8, name="xa", tag="xa")
                PART = NKT * T // 4
                for part in range(4):
                    nc.sync.dma_start(
                        xa[:, part * PART:(part + 1) * PART],
                        xt_ap[:, mt * NKT * T + part * PART:
                              mt * NKT * T + (part + 1) * PART])
                if mt == 0:
                    emit_const_loads()

                s_sb = emit_scores(xa)
                stn_all = emit_transpose_scores(s_sb)
                if prev is not None:
                    prev["gt"] = emit_transpose_gates(prev["ggs"])

                da = dpool.tile([128, NKT * T], f8, name="da", tag="da")
                cur = {"mt": mt, "da": da, "zbs": [], "ggs": None}

                for q in range(4):
                    vq = {1, 3} if q % 2 else {1}
                    if prev is not None:
                        emit_gx_mul(prev, q)
                        emit_delta_pairs(prev, q, (0,), veceng=vq)
                    zp = pp.tile([128, T], f32, space="PSUM", name="zp",
                                 tag="zp")
                    for part in range(4):
                        emit_z_part(xa, q, part, zp)
                        if prev is not None and part < 3:
                            emit_delta_pairs(prev, q, (part + 1,), veceng=vq)
                    if prev is not None:
                        emit_store(prev, q)
                    zb = zpool.tile([128, T], bf16, name="zb", tag="zb")
                    nc.scalar.activation(zb[:], zp[:], AF.Identity,
                                         bias=bias[:, q:q + 1], scale=1.0)
                    cur["zbs"].append(zb)
                    if mt == NMT - 1:
                        # last macrotile: its chain competes with the tail,
                        # so spread the sub-chains across the z quarters
                        lastggs.append(emit_chain_ts(stn_all, q))

                if mt == NMT - 1:
                    cur["ggs"] = lastggs
                else:
                    cur["ggs"] = emit_chain(stn_all)
                prev = cur

            # ---- drain the pipeline: delta of the last macrotile ----
            prev["gt"] = emit_transpose_gates(prev["ggs"])
            # tail drain: all 4 gate-expand matmuls + zg muls run first
            # (each mul frees the single gx bank; the vector queue is empty
            # so the round-trips are short), then the delta matmul stream
            # runs without any vector-queue interruptions.  The z-phase
            # "zp" PSUM tiles are dead now, so the delta chunks cycle
            # through dp pairs AND zp singles (6 chunks in flight).
            for q in range(4):
                emit_gx_mul(prev, q)
            for q in range(4):
                zgq = prev["zbs"][q]
                for grp in range(2):           # chunks [0..3] then [4..7]
                    hc0 = q * 8 + grp * 4
                    dp = pp.tile([128, 2 * T], f32, space="PSUM", name="dp",
                                 tag="dp", bufs=2)
                    for hf in range(2):
                        nc.tensor.matmul(
                            dp[:, hf * T:(hf + 1) * T],
                            bpk[:, (hc0 + hf) * 128:(hc0 + hf + 1) * 128],
                            zgq[:], start=True, stop=True)
                    nc.scalar.copy(prev["da"][:, hc0 * T:(hc0 + 1) * T],
                                   dp[:, 0:T])
                    nc.vector.tensor_copy(
                        prev["da"][:, (hc0 + 1) * T:(hc0 + 2) * T],
                        dp[:, T:2 * T])
                    for sng in range(2):
                        hc = hc0 + 2 + sng
                        zps = pp.tile([128, T], f32, space="PSUM", name="zp",
                                      tag="zp")
                        nc.tensor.matmul(zps[:],
                                         bpk[:, hc * 128:(hc + 1) * 128],
                                         zgq[:], start=True, stop=True)
                        dsl = prev["da"][:, hc * T:(hc + 1) * T]
                        if sng == 0:
                            nc.scalar.copy(dsl, zps[:])
                        else:
                            nc.vector.tensor_copy(dsl, zps[:])
                emit_store(prev, q)

    nc.compile()
    return nc


def _prep_consts(task_emb, task_ids, Wp, bp, centers, A, Bm, adapter_scale):
    scale = float(np.asarray(adapter_scale))
    A_all = np.ascontiguousarray(
        A.transpose(1, 0, 2).reshape(H, NB * R).astype(np.float32))
    W2 = (Wp @ centers.T).astype(np.float32)                     # [H, 128]

    # ah: [p, q, k2, two, m] = A_all[(2*k2+two)*128+p, q*128+m], fp8 e4m3
    # (DoubleRow pairs of consecutive k-tiles interleave along the free dim)
    ah = (A_all.reshape(NKT, 128, 4, 128).transpose(1, 2, 0, 3)
          .reshape(128, 4 * NKT * 128).astype(ml_dtypes.float8_e4m3))
    ah = np.ascontiguousarray(ah)
    # ws: [p, k2, two, m] = W2[(2*k2+two)*128+p, m], fp8 (DoubleRow pairs)
    wsn = np.ascontiguousarray(
        W2.reshape(NKT, 128, 128).transpose(1, 0, 2).reshape(128, NKT * 128)
        .astype(ml_dtypes.float8_e4m3))

    # block-diag up-projection, K=128 per h-chunk
    bpk = np.zeros((128, NKT * 128), np.float32)
    for hc in range(NKT):
        for mblk in range(4):
            n = hc * 4 + mblk
            for r in range(R):
                row = (hc % 8) * 16 + mblk * 4 + r
                bpk[row, hc * 128 + mblk * 32: hc * 128 + mblk * 32 + 32] = \
                    Bm[n, r, :] * scale
    bpk = bpk.astype(ml_dtypes.bfloat16)

    e_np = (np.arange(128)[:, None] == (np.arange(512)[None, :] // 4)) \
        .astype(ml_dtypes.bfloat16)

    sconst = (bp @ centers.T - 0.5 * (centers ** 2).sum(-1)).astype(np.float32)

    biases = []
    for c in range(NCORES):
        te = task_emb[int(np.asarray(task_ids)[c // 2])].astype(np.float32)
        b5 = np.empty((128, 5), np.float32)
        zoff = te @ A_all                                        # [512]
        for q in range(4):
            b5[:, q] = zoff[q * 128:(q + 1) * 128]
        b5[:, 4] = te @ W2 + sconst
        biases.append(np.ascontiguousarray(b5))
    return ah, wsn, bpk, e_np, biases


def kernel(x, task_ids, task_emb, Wp, bp, centers, A, Bm, adapter_scale):
    global _COMPILED, LAST_RESULT
    from concourse import bass_utils

    x = np.asarray(x, dtype=np.float32)
    task_ids = np.asarray(task_ids)
    task_emb = np.asarray(task_emb, dtype=np.float32)
    Wp = np.asarray(Wp, dtype=np.float32)
    bp = np.asarray(bp, dtype=np.float32)
    centers = np.asarray(centers, dtype=np.float32)
    A = np.asarray(A, dtype=np.float32)
    Bm = np.asarray(Bm, dtype=np.float32)

    if _COMPILED is None:
        _COMPILED = _build()
    nc = _COMPILED

    ah, wsn, bpk, e_np, biases = _prep_consts(
        task_emb, task_ids, Wp, bp, centers, A, Bm, adapter_scale)

    xf = x.reshape(B * S, H)
    xf8 = xf.astype(ml_dtypes.float8_e4m3)
    in_maps = []
    for c in range(NCORES):
        xtc = xf8[c * TPC:(c + 1) * TPC].reshape(NMT, T, NKT, 128)
        xtc = np.ascontiguousarray(xtc.transpose(3, 0, 2, 1)) \
            .reshape(128, NMT * NKT * T)
        in_maps.append({"xt": xtc, "ah": ah, "ws": wsn, "bpk": bpk,
                       "e": e_np, "bias": biases[c]})

    kwargs = {}
    if TRACE:
        kwargs = dict(trace=True, tmpdir=TRACE_DIR)
    res = bass_utils.run_bass_kernel_spmd(
        nc, in_maps, core_ids=list(range(NCORES)), **kwargs)
    LAST_RESULT = res

    out = np.empty((B * S, H), np.float32)
    for c in range(NCORES):
        dat = res.results[c]["yt"].reshape(128, NMT, 4, 8, T)
        delta = dat.transpose(1, 4, 2, 3, 0).reshape(TPC, H)
        out[c * TPC:(c + 1) * TPC] = xf[c * TPC:(c + 1) * TPC] + \
            delta.astype(np.float32)
    return out.reshape(B, S, H)


# revision 23
# speedup vs baseline: 1.1575x; 1.0022x over previous
"""Trainium2 Bass kernel for NeuroplasticLlama block-sparse adapter (moe_routing).

Contract: kernel(**inputs) takes FULL unsharded inputs (as produced by
setup_inputs) and returns the FULL [4, 4096, 4096] float32 output.

Strategy (data/sequence parallel over 8 cores, 2048 tokens each):
  - Each core's 2048 contiguous tokens belong to exactly one batch, so the
    task embedding contributes only per-core constant bias vectors
    (te @ A, te @ W2) -- h = x + te is never materialized.
  - Dense formulation of the routed computation:
      scores s[t,n] = x @ (Wp @ centers.T)[:,n] + const_n (shift dropped;
        top-k set and softmax are shift invariant)
      top-3 via threshold = 3rd max (3 rounds of max + mask-out)
      gates g[t,n] = exp(s - max) * (s >= thr3) / sum(...)
      z[t,:] = x @ A_all (all 512 block-rank pairs, dense)
      zg = z * expand4(g);  delta = block-diag(Bm) matmul
  - The device computes only DELTA (fp8 in, fp8 out); the residual
    y = x + delta is applied on the host during unsharding, keeping the
    x term exact and HBM traffic at 2 MB in + 2 MB out per 512-token
    macrotile per core.
  - x is fed pre-transposed and pre-shuffled to the [p][mt][k][t] layout
    so every DMA is a contiguous [128, N] transfer (128 descriptors; a
    strided [H, tokens] slice costs ~18us of descriptor generation).
  - scores and z are fp8 DoubleRow matmuls off the same x tile; delta is
    bf16 (output-stream bound, DoubleRow would not help).
  - SOFTWARE PIPELINE: macrotile m's delta/gate-apply phase executes
    during macrotile m+1's scores/z phase, with delta pair-matmuls
    interleaved between z chunks.  The PSUM->SBUF delta drains (the
    throughput-critical ~9us/mt of scalar+vector work) then overlap the
    z matmuls instead of bursting at the end of each macrotile, and the
    gating chain of m+1 runs in the vector queue after m's drains.
"""

import sys

if "/opt/trn_rl_repo" not in sys.path:
    sys.path.insert(0, "/opt/trn_rl_repo")

import numpy as np
import ml_dtypes

H = 4096
NB = 128
BLK = 32
R = 4
B = 4
S = 4096
NCORES = 8
TPC = (B * S) // NCORES  # tokens per core = 2048
T = 512                  # tokens per macrotile
NMT = TPC // T           # 4 macrotiles per core
NKT = H // 128           # 32 k-tiles over the hidden dim
NP = NKT // 2            # 16 DoubleRow k-pair tiles
BIG = 1.0e30

TRACE = False            # set by test.py for profiling runs
TRACE_DIR = None
LAST_RESULT = None       # BassKernelResults of the last run

_COMPILED = None


def _build():
    import concourse.bacc as bacc
    import concourse.tile as tile
    from concourse import mybir, masks

    f32 = mybir.dt.float32
    bf16 = mybir.dt.bfloat16
    f8 = mybir.dt.float8e4
    AF = mybir.ActivationFunctionType
    AL = mybir.AluOpType
    AX = mybir.AxisListType
    DR = mybir.MatmulPerfMode.DoubleRow

    nc = bacc.Bacc("TRN2", target_bir_lowering=False, debug=False,
                   num_devices=NCORES)

    xt_d = nc.dram_tensor("xt", [128, NMT * NKT * T], f8, kind="ExternalInput")
    ah_d = nc.dram_tensor("ah", [128, 4 * NKT * 128], f8, kind="ExternalInput")
    ws_d = nc.dram_tensor("ws", [128, NKT * 128], f8, kind="ExternalInput")
    bpk_d = nc.dram_tensor("bpk", [128, NKT * 128], bf16, kind="ExternalInput")
    e_d = nc.dram_tensor("e", [128, 512], bf16, kind="ExternalInput")
    bias_d = nc.dram_tensor("bias", [128, 5], f32, kind="ExternalInput")
    yt_d = nc.dram_tensor("yt", [128, NMT * NKT * T], f8, kind="ExternalOutput")

    xt_ap = xt_d.ap()
    yt_ap = yt_d.ap()

    with tile.TileContext(nc) as tc:
        from contextlib import ExitStack
        with ExitStack() as ctx:
            cpool = ctx.enter_context(tc.tile_pool(name="consts", bufs=1))
            xpool = ctx.enter_context(tc.tile_pool(name="xg", bufs=2))
            dpool = ctx.enter_context(tc.tile_pool(name="dall", bufs=2))
            zpool = ctx.enter_context(tc.tile_pool(name="zb", bufs=8))
            gpool = ctx.enter_context(tc.tile_pool(name="gate", bufs=3))
            spool = ctx.enter_context(tc.tile_pool(name="scal", bufs=4))
            pp = ctx.enter_context(tc.tile_pool(name="ps", bufs=2, space="PSUM"))

            # ---- persistent constants ----
            # xa0 is issued FIRST (in the macrotile loop below) on the sync
            # HWDGE ring; az/bpk/esb queue BEHIND it on the same ring so the
            # first scores matmul's input gets the full HBM bandwidth
            # (per-queue FIFO), while the tiny ws/bias go in parallel on
            # scalar/gpsimd.  az0 is only needed ~4us after scores start.
            ws = cpool.tile([128, NKT * 128], f8, name="ws", tag="ws")
            nc.scalar.dma_start(ws[:, 0:8 * 256], ws_d.ap()[:, 0:8 * 256])
            nc.scalar.dma_start(ws[:, 8 * 256:], ws_d.ap()[:, 8 * 256:])
            bias = cpool.tile([128, 5], f32, name="bias", tag="bias")
            nc.gpsimd.dma_start(bias[:], bias_d.ap()[:])
            ident = cpool.tile([128, 128], bf16, name="ident", tag="ident")
            masks.make_identity(nc, ident[:])
            kshift = cpool.tile([128, 1], f32, name="kshift", tag="kshift")
            nc.gpsimd.memset(kshift[:], 1000.0)
            az = []
            bpk = cpool.tile([128, NKT * 128], bf16, name="bpk", tag="bpk")
            esb = cpool.tile([128, 512], bf16, name="esb", tag="esb")

            def emit_const_loads():
                for q in range(4):
                    t_az = cpool.tile([128, NKT * 128], f8, name=f"az{q}",
                                      tag=f"az{q}")
                    nc.sync.dma_start(
                        t_az[:],
                        ah_d.ap()[:, q * NKT * 128:(q + 1) * NKT * 128])
                    az.append(t_az)
                nc.sync.dma_start(bpk[:], bpk_d.ap()[:])
                nc.sync.dma_start(esb[:], e_d.ap()[:])

            NTS = T // 128  # token sub-tiles per macrotile

            # ---------------- pipelined stage helpers ----------------
            def emit_scores(xa):
                sp = pp.tile([128, T], f32, space="PSUM", name="sp", tag="zp")
                for k2 in range(NP):
                    nc.tensor.matmul(
                        sp[:],
                        ws[:, k2 * 256:(k2 + 1) * 256]
                        .rearrange("p (two m) -> p two m", two=2),
                        xa[:, 2 * k2 * T:(2 * k2 + 2) * T]
                        .rearrange("p (two t) -> p two t", two=2),
                        start=(k2 == 0), stop=(k2 == NP - 1),
                        perf_mode=DR,
                    )
                s_sb = gpool.tile([128, T], bf16, name="s_sb", tag="s_sb")
                nc.scalar.activation(s_sb[:], sp[:], AF.Identity,
                                     bias=bias[:, 4:5], scale=1.0)
                return s_sb

            def emit_transpose_scores(s_sb):
                s_ps = pp.tile([128, T], bf16, space="PSUM", name="s_ps",
                               tag="tr", bufs=1)
                for ts in range(NTS):
                    nc.tensor.transpose(s_ps[:, ts * 128:(ts + 1) * 128],
                                        s_sb[:, ts * 128:(ts + 1) * 128],
                                        ident[:])
                stn_all = gpool.tile([128, T], f32, name="stn_all", tag="stn",
                                     bufs=2)
                # +1000 shift makes every score positive, so the chain can
                # mask out maxima with s*(s<r) in one fused op per round
                nc.scalar.activation(stn_all[:], s_ps[:], AF.Identity,
                                     bias=kshift[:], scale=1.0)
                return stn_all

            def emit_transpose_gates(ggs):
                g_ps = pp.tile([128, T], bf16, space="PSUM", name="g_ps",
                               tag="tr", bufs=1)
                for ts in range(NTS):
                    nc.tensor.transpose(g_ps[:, ts * 128:(ts + 1) * 128],
                                        ggs[ts][:], ident[:])
                gt_sb = gpool.tile([128, T], bf16, name="gt_sb", tag="gt_sb")
                nc.scalar.copy(gt_sb[:], g_ps[:])
                return gt_sb

            def emit_z_part(xa, q, part, zp):
                # one quarter of the k2 range (4 DoubleRow matmuls)
                for k2 in range(NP // 4 * part, NP // 4 * (part + 1)):
                    nc.tensor.matmul(
                        zp[:],
                        az[q][:, k2 * 256:(k2 + 1) * 256]
                        .rearrange("p (two m) -> p two m", two=2),
                        xa[:, 2 * k2 * T:(2 * k2 + 2) * T]
                        .rearrange("p (two t) -> p two t", two=2),
                        start=(k2 == 0), stop=(k2 == NP - 1),
                        perf_mode=DR,
                    )

            def emit_gx_mul(st, q):
                gx = pp.tile([128, T], f32, space="PSUM", name="gx", tag="gx",
                             bufs=1)
                nc.tensor.matmul(gx[:],
                                 esb[:, q * 128:(q + 1) * 128],
                                 st["gt"][:],
                                 start=True, stop=True)
                nc.vector.tensor_mul(st["zbs"][q][:], st["zbs"][q][:], gx[:])

            def emit_delta_pairs(st, q, prs, veceng):
                # delta pair matmuls + PSUM->SBUF fp8 drains for pair
                # indices prs of quarter q of the previous macrotile
                for pr in prs:
                    hc = q * 8 + pr * 2
                    dp = pp.tile([128, 2 * T], f32, space="PSUM", name="dp",
                                 tag="dp", bufs=2)
                    for hf in range(2):
                        nc.tensor.matmul(
                            dp[:, hf * T:(hf + 1) * T],
                            bpk[:, (hc + hf) * 128:(hc + hf + 1) * 128],
                            st["zbs"][q][:],
                            start=True, stop=True)
                    if veceng == "both":
                        # tail drain: no later z-phase to hide behind, so
                        # both engines drain one half of every pair
                        nc.scalar.copy(st["da"][:, hc * T:(hc + 1) * T],
                                       dp[:, 0:T])
                        nc.vector.tensor_copy(
                            st["da"][:, (hc + 1) * T:(hc + 2) * T],
                            dp[:, T:2 * T])
                    elif pr in veceng:
                        nc.vector.tensor_copy(st["da"][:, hc * T:(hc + 2) * T],
                                              dp[:])
                    else:
                        nc.scalar.copy(st["da"][:, hc * T:(hc + 2) * T],
                                       dp[:])

            def emit_store(st, q):
                smt = st["mt"]
                nc.sync.dma_start(
                    yt_ap[:, (smt * 4 + q) * 8 * T:(smt * 4 + q + 1) * 8 * T],
                    st["da"][:, q * 8 * T:(q + 1) * 8 * T],
                )

            def emit_chain(stn_all, only_ts=None):
                # the DVE gating chain for token sub-tiles
                ggs = []
                tslist = range(NTS) if only_ts is None else [only_ts]
                for ts in tslist:
                    stn = stn_all[:, ts * 128:(ts + 1) * 128]
                    r1 = spool.tile([128, 1], f32, name="r1", tag="r1")
                    nc.vector.reduce_max(r1[:], stn, axis=AX.X)
                    s2 = gpool.tile([128, 128], f32, name="s2", tag="s2")
                    nc.vector.scalar_tensor_tensor(s2[:], stn, r1[:], stn,
                                                   AL.is_lt, AL.mult)
                    r2 = spool.tile([128, 1], f32, name="r2", tag="r2")
                    nc.vector.reduce_max(r2[:], s2[:], axis=AX.X)
                    s3 = gpool.tile([128, 128], f32, name="s3", tag="s3")
                    nc.vector.scalar_tensor_tensor(s3[:], s2[:], r2[:], s2[:],
                                                   AL.is_lt, AL.mult)
                    r3 = spool.tile([128, 1], f32, name="r3", tag="r3")
                    nc.vector.reduce_max(r3[:], s3[:], axis=AX.X)
                    nr1 = spool.tile([128, 1], f32, name="nr1", tag="nr1")
                    nc.vector.tensor_scalar_mul(nr1[:], r1[:], -1.0)
                    ex = gpool.tile([128, 128], f32, name="ex", tag="ex")
                    nc.scalar.activation(ex[:], stn, AF.Exp, bias=nr1[:],
                                         scale=1.0)
                    em = gpool.tile([128, 128], f32, name="em", tag="em")
                    zs = spool.tile([128, 1], f32, name="zs", tag="zs")
                    nc.vector.scalar_tensor_tensor(em[:], stn, r3[:], ex[:],
                                                   AL.is_ge, AL.mult,
                                                   accum_out=zs[:])
                    rz = spool.tile([128, 1], f32, name="rz", tag="rz")
                    nc.vector.reciprocal(rz[:], zs[:])
                    gg = gpool.tile([128, 128], bf16, name="gg", tag="gg",
                                    bufs=NTS + 1)
                    nc.vector.tensor_scalar_mul(gg[:], em[:], rz[:])
                    ggs.append(gg)
                return ggs

            def emit_chain_ts(stn_all, ts):
                return emit_chain(stn_all, only_ts=ts)[0]

            # ---------------- the pipelined macrotile loop ----------------
            prev = None  # state of the previous macrotile
            lastggs = []
            for mt in range(NMT):
                xa = xpool.tile([128, NKT * T], f8, name="xa", tag="xa")
                PART = NKT * T // 4
                for part in range(4):
                    nc.sync.dma_start(
                        xa[:, part * PART:(part + 1) * PART],
                        xt_ap[:, mt * NKT * T + part * PART:
                              mt * NKT * T + (part + 1) * PART])
                if mt == 0:
                    emit_const_loads()

                s_sb = emit_scores(xa)
                stn_all = emit_transpose_scores(s_sb)
                if prev is not None:
                    prev["gt"] = emit_transpose_gates(prev["ggs"])

                da = dpool.tile([128, NKT * T], f8, name="da", tag="da")
                cur = {"mt": mt, "da": da, "zbs": [], "ggs": None}

                for q in range(4):
                    vq = {1, 3} if q % 2 else {1}
                    if prev is not None:
                        emit_gx_mul(prev, q)
                        emit_delta_pairs(prev, q, (0,), veceng=vq)
                    zp = pp.tile([128, T], f32, space="PSUM", name="zp",
                                 tag="zp")
                    for part in range(4):
                        emit_z_part(xa, q, part, zp)
                        if prev is not None and part < 3:
                            emit_delta_pairs(prev, q, (part + 1,), veceng=vq)
                    if prev is not None:
                        emit_store(prev, q)
                    zb = zpool.tile([128, T], bf16, name="zb", tag="zb")
                    nc.scalar.activation(zb[:], zp[:], AF.Identity,
                                         bias=bias[:, q:q + 1], scale=1.0)
                    cur["zbs"].append(zb)
                    if mt == NMT - 1:
                        # last macrotile: its chain competes with the tail,
                        # so spread the sub-chains across the z quarters
                        lastggs.append(emit_chain_ts(stn_all, q))

                if mt == NMT - 1:
                    cur["ggs"] = lastggs
                else:
                    cur["ggs"] = emit_chain(stn_all)
                prev = cur

            # ---- drain the pipeline: delta of the last macrotile ----
            prev["gt"] = emit_transpose_gates(prev["ggs"])
            # tail drain: all 4 gate-expand matmuls + zg muls run first
            # (each mul frees the single gx bank; the vector queue is empty
            # so the round-trips are short), then the delta matmul stream
            # runs without any vector-queue interruptions.  The z-phase
            # "zp" PSUM tiles are dead now, so the delta chunks cycle
            # through dp pairs AND zp singles (6 chunks in flight).
            for q in range(4):
                emit_gx_mul(prev, q)
            for q in range(4):
                zgq = prev["zbs"][q]
                for grp in range(2):           # chunks [0..3] then [4..7]
                    hc0 = q * 8 + grp * 4
                    dp = pp.tile([128, 2 * T], f32, space="PSUM", name="dp",
                                 tag="dp", bufs=2)
                    for hf in range(2):
                        nc.tensor.matmul(
                            dp[:, hf * T:(hf + 1) * T],
                            bpk[:, (hc0 + hf) * 128:(hc0 + hf + 1) * 128],
                            zgq[:], start=True, stop=True)
                    nc.scalar.copy(prev["da"][:, hc0 * T:(hc0 + 1) * T],
                                   dp[:, 0:T])
                    nc.vector.tensor_copy(
                        prev["da"][:, (hc0 + 1) * T:(hc0 + 2) * T],
                        dp[:, T:2 * T])
                    for sng in range(2):
                        hc = hc0 + 2 + sng
                        zps = pp.tile([128, T], f32, space="PSUM", name="zp",
                                      tag="zp")
                        nc.tensor.matmul(zps[:],
                                         bpk[:, hc * 128:(hc + 1) * 128],
                                         zgq[:], start=True, stop=True)
                        dsl = prev["da"][:, hc * T:(hc + 1) * T]
                        if sng == 0:
                            nc.scalar.copy(dsl, zps[:])
                        else:
                            nc.vector.tensor_copy(dsl, zps[:])
                    # store each 4-chunk group as soon as it drains so the
                    # final store (and its ~2us completion latency) is small
                    smt = prev["mt"]
                    off = (smt * 4 + q) * 8 * T + grp * 4 * T
                    nc.sync.dma_start(
                        yt_ap[:, off:off + 4 * T],
                        prev["da"][:, hc0 * T:(hc0 + 4) * T],
                    )

    nc.compile()
    return nc


def _prep_consts(task_emb, task_ids, Wp, bp, centers, A, Bm, adapter_scale):
    scale = float(np.asarray(adapter_scale))
    A_all = np.ascontiguousarray(
        A.transpose(1, 0, 2).reshape(H, NB * R).astype(np.float32))
    W2 = (Wp @ centers.T).astype(np.float32)                     # [H, 128]

    # ah: [p, q, k2, two, m] = A_all[(2*k2+two)*128+p, q*128+m], fp8 e4m3
    # (DoubleRow pairs of consecutive k-tiles interleave along the free dim)
    ah = (A_all.reshape(NKT, 128, 4, 128).transpose(1, 2, 0, 3)
          .reshape(128, 4 * NKT * 128).astype(ml_dtypes.float8_e4m3))
    ah = np.ascontiguousarray(ah)
    # ws: [p, k2, two, m] = W2[(2*k2+two)*128+p, m], fp8 (DoubleRow pairs)
    wsn = np.ascontiguousarray(
        W2.reshape(NKT, 128, 128).transpose(1, 0, 2).reshape(128, NKT * 128)
        .astype(ml_dtypes.float8_e4m3))

    # block-diag up-projection, K=128 per h-chunk
    bpk = np.zeros((128, NKT * 128), np.float32)
    for hc in range(NKT):
        for mblk in range(4):
            n = hc * 4 + mblk
            for r in range(R):
                row = (hc % 8) * 16 + mblk * 4 + r
                bpk[row, hc * 128 + mblk * 32: hc * 128 + mblk * 32 + 32] = \
                    Bm[n, r, :] * scale
    bpk = bpk.astype(ml_dtypes.bfloat16)

    e_np = (np.arange(128)[:, None] == (np.arange(512)[None, :] // 4)) \
        .astype(ml_dtypes.bfloat16)

    sconst = (bp @ centers.T - 0.5 * (centers ** 2).sum(-1)).astype(np.float32)

    biases = []
    for c in range(NCORES):
        te = task_emb[int(np.asarray(task_ids)[c // 2])].astype(np.float32)
        b5 = np.empty((128, 5), np.float32)
        zoff = te @ A_all                                        # [512]
        for q in range(4):
            b5[:, q] = zoff[q * 128:(q + 1) * 128]
        b5[:, 4] = te @ W2 + sconst
        biases.append(np.ascontiguousarray(b5))
    return ah, wsn, bpk, e_np, biases


def kernel(x, task_ids, task_emb, Wp, bp, centers, A, Bm, adapter_scale):
    global _COMPILED, LAST_RESULT
    from concourse import bass_utils

    x = np.asarray(x, dtype=np.float32)
    task_ids = np.asarray(task_ids)
    task_emb = np.asarray(task_emb, dtype=np.float32)
    Wp = np.asarray(Wp, dtype=np.float32)
    bp = np.asarray(bp, dtype=np.float32)
    centers = np.asarray(centers, dtype=np.float32)
    A = np.asarray(A, dtype=np.float32)
    Bm = np.asarray(Bm, dtype=np.float32)

    if _COMPILED is None:
        _COMPILED = _build()
    nc = _COMPILED

    ah, wsn, bpk, e_np, biases = _prep_consts(
        task_emb, task_ids, Wp, bp, centers, A, Bm, adapter_scale)

    xf = x.reshape(B * S, H)
    xf8 = xf.astype(ml_dtypes.float8_e4m3)
    in_maps = []
    for c in range(NCORES):
        xtc = xf8[c * TPC:(c + 1) * TPC].reshape(NMT, T, NKT, 128)
        xtc = np.ascontiguousarray(xtc.transpose(3, 0, 2, 1)) \
            .reshape(128, NMT * NKT * T)
        in_maps.append({"xt": xtc, "ah": ah, "ws": wsn, "bpk": bpk,
                       "e": e_np, "bias": biases[c]})

    kwargs = {}
    if TRACE:
        kwargs = dict(trace=True, tmpdir=TRACE_DIR)
    res = bass_utils.run_bass_kernel_spmd(
        nc, in_maps, core_ids=list(range(NCORES)), **kwargs)
    LAST_RESULT = res

    out = np.empty((B * S, H), np.float32)
    for c in range(NCORES):
        dat = res.results[c]["yt"].reshape(128, NMT, 4, 8, T)
        delta = dat.transpose(1, 4, 2, 3, 0).reshape(TPC, H)
        out[c * TPC:(c + 1) * TPC] = xf[c * TPC:(c + 1) * TPC] + \
            delta.astype(np.float32)
    return out.reshape(B, S, H)


# revision 24
# speedup vs baseline: 1.1610x; 1.0030x over previous
"""Trainium2 Bass kernel for NeuroplasticLlama block-sparse adapter (moe_routing).

Contract: kernel(**inputs) takes FULL unsharded inputs (as produced by
setup_inputs) and returns the FULL [4, 4096, 4096] float32 output.

Strategy (data/sequence parallel over 8 cores, 2048 tokens each):
  - Each core's 2048 contiguous tokens belong to exactly one batch, so the
    task embedding contributes only per-core constant bias vectors
    (te @ A, te @ W2) -- h = x + te is never materialized.
  - Dense formulation of the routed computation:
      scores s[t,n] = x @ (Wp @ centers.T)[:,n] + const_n (shift dropped;
        top-k set and softmax are shift invariant)
      top-3 via threshold = 3rd max (3 rounds of max + mask-out)
      gates g[t,n] = exp(s - max) * (s >= thr3) / sum(...)
      z[t,:] = x @ A_all (all 512 block-rank pairs, dense)
      zg = z * expand4(g);  delta = block-diag(Bm) matmul
  - The device computes only DELTA (fp8 in, fp8 out); the residual
    y = x + delta is applied on the host during unsharding, keeping the
    x term exact and HBM traffic at 2 MB in + 2 MB out per 512-token
    macrotile per core.
  - x is fed pre-transposed and pre-shuffled to the [p][mt][k][t] layout
    so every DMA is a contiguous [128, N] transfer (128 descriptors; a
    strided [H, tokens] slice costs ~18us of descriptor generation).
  - scores and z are fp8 DoubleRow matmuls off the same x tile; delta is
    bf16 (output-stream bound, DoubleRow would not help).
  - SOFTWARE PIPELINE: macrotile m's delta/gate-apply phase executes
    during macrotile m+1's scores/z phase, with delta pair-matmuls
    interleaved between z chunks.  The PSUM->SBUF delta drains (the
    throughput-critical ~9us/mt of scalar+vector work) then overlap the
    z matmuls instead of bursting at the end of each macrotile, and the
    gating chain of m+1 runs in the vector queue after m's drains.
"""

import sys

if "/opt/trn_rl_repo" not in sys.path:
    sys.path.insert(0, "/opt/trn_rl_repo")

import numpy as np
import ml_dtypes

H = 4096
NB = 128
BLK = 32
R = 4
B = 4
S = 4096
NCORES = 8
TPC = (B * S) // NCORES  # tokens per core = 2048
T = 512                  # tokens per macrotile
NMT = TPC // T           # 4 macrotiles per core
NKT = H // 128           # 32 k-tiles over the hidden dim
NP = NKT // 2            # 16 DoubleRow k-pair tiles
BIG = 1.0e30

TRACE = False            # set by test.py for profiling runs
TRACE_DIR = None
LAST_RESULT = None       # BassKernelResults of the last run

_COMPILED = None


def _build():
    import concourse.bacc as bacc
    import concourse.tile as tile
    from concourse import mybir, masks

    f32 = mybir.dt.float32
    bf16 = mybir.dt.bfloat16
    f8 = mybir.dt.float8e4
    AF = mybir.ActivationFunctionType
    AL = mybir.AluOpType
    AX = mybir.AxisListType
    DR = mybir.MatmulPerfMode.DoubleRow

    nc = bacc.Bacc("TRN2", target_bir_lowering=False, debug=False,
                   num_devices=NCORES)

    xt_d = nc.dram_tensor("xt", [128, NMT * NKT * T], f8, kind="ExternalInput")
    ah_d = nc.dram_tensor("ah", [128, 4 * NKT * 128], f8, kind="ExternalInput")
    ws_d = nc.dram_tensor("ws", [128, NKT * 128], f8, kind="ExternalInput")
    bpk_d = nc.dram_tensor("bpk", [128, NKT * 128], bf16, kind="ExternalInput")
    e_d = nc.dram_tensor("e", [128, 512], bf16, kind="ExternalInput")
    bias_d = nc.dram_tensor("bias", [128, 5], f32, kind="ExternalInput")
    yt_d = nc.dram_tensor("yt", [128, NMT * NKT * T], f8, kind="ExternalOutput")

    xt_ap = xt_d.ap()
    yt_ap = yt_d.ap()

    with tile.TileContext(nc) as tc:
        from contextlib import ExitStack
        with ExitStack() as ctx:
            cpool = ctx.enter_context(tc.tile_pool(name="consts", bufs=1))
            xpool = ctx.enter_context(tc.tile_pool(name="xg", bufs=2))
            dpool = ctx.enter_context(tc.tile_pool(name="dall", bufs=2))
            zpool = ctx.enter_context(tc.tile_pool(name="zb", bufs=8))
            gpool = ctx.enter_context(tc.tile_pool(name="gate", bufs=3))
            spool = ctx.enter_context(tc.tile_pool(name="scal", bufs=4))
            pp = ctx.enter_context(tc.tile_pool(name="ps", bufs=2, space="PSUM"))

            # ---- persistent constants ----
            # xa0 is issued FIRST (in the macrotile loop below) on the sync
            # HWDGE ring; az/bpk/esb queue BEHIND it on the same ring so the
            # first scores matmul's input gets the full HBM bandwidth
            # (per-queue FIFO), while the tiny ws/bias go in parallel on
            # scalar/gpsimd.  az0 is only needed ~4us after scores start.
            ws = cpool.tile([128, NKT * 128], f8, name="ws", tag="ws")
            nc.scalar.dma_start(ws[:, 0:8 * 256], ws_d.ap()[:, 0:8 * 256])
            nc.scalar.dma_start(ws[:, 8 * 256:], ws_d.ap()[:, 8 * 256:])
            bias = cpool.tile([128, 5], f32, name="bias", tag="bias")
            nc.gpsimd.dma_start(bias[:], bias_d.ap()[:])
            ident = cpool.tile([128, 128], bf16, name="ident", tag="ident")
            masks.make_identity(nc, ident[:])
            kshift = cpool.tile([128, 1], f32, name="kshift", tag="kshift")
            nc.gpsimd.memset(kshift[:], 1000.0)
            warmt = cpool.tile([128, 512], bf16, name="warmt", tag="warmt")
            nc.gpsimd.memset(warmt[:], 0.0)
            az = []
            bpk = cpool.tile([128, NKT * 128], bf16, name="bpk", tag="bpk")
            esb = cpool.tile([128, 512], bf16, name="esb", tag="esb")

            def emit_const_loads():
                for q in range(4):
                    t_az = cpool.tile([128, NKT * 128], f8, name=f"az{q}",
                                      tag=f"az{q}")
                    nc.sync.dma_start(
                        t_az[:],
                        ah_d.ap()[:, q * NKT * 128:(q + 1) * NKT * 128])
                    az.append(t_az)
                nc.sync.dma_start(bpk[:], bpk_d.ap()[:])
                nc.sync.dma_start(esb[:], e_d.ap()[:])

            NTS = T // 128  # token sub-tiles per macrotile

            # ---------------- pipelined stage helpers ----------------
            def emit_scores(xa):
                sp = pp.tile([128, T], f32, space="PSUM", name="sp", tag="zp")
                for k2 in range(NP):
                    nc.tensor.matmul(
                        sp[:],
                        ws[:, k2 * 256:(k2 + 1) * 256]
                        .rearrange("p (two m) -> p two m", two=2),
                        xa[:, 2 * k2 * T:(2 * k2 + 2) * T]
                        .rearrange("p (two t) -> p two t", two=2),
                        start=(k2 == 0), stop=(k2 == NP - 1),
                        perf_mode=DR,
                    )
                s_sb = gpool.tile([128, T], bf16, name="s_sb", tag="s_sb")
                nc.scalar.activation(s_sb[:], sp[:], AF.Identity,
                                     bias=bias[:, 4:5], scale=1.0)
                return s_sb

            def emit_transpose_scores(s_sb):
                s_ps = pp.tile([128, T], bf16, space="PSUM", name="s_ps",
                               tag="tr", bufs=1)
                for ts in range(NTS):
                    nc.tensor.transpose(s_ps[:, ts * 128:(ts + 1) * 128],
                                        s_sb[:, ts * 128:(ts + 1) * 128],
                                        ident[:])
                stn_all = gpool.tile([128, T], f32, name="stn_all", tag="stn",
                                     bufs=2)
                # +1000 shift makes every score positive, so the chain can
                # mask out maxima with s*(s<r) in one fused op per round
                nc.scalar.activation(stn_all[:], s_ps[:], AF.Identity,
                                     bias=kshift[:], scale=1.0)
                return stn_all

            def emit_transpose_gates(ggs):
                g_ps = pp.tile([128, T], bf16, space="PSUM", name="g_ps",
                               tag="tr", bufs=1)
                for ts in range(NTS):
                    nc.tensor.transpose(g_ps[:, ts * 128:(ts + 1) * 128],
                                        ggs[ts][:], ident[:])
                gt_sb = gpool.tile([128, T], bf16, name="gt_sb", tag="gt_sb")
                nc.scalar.copy(gt_sb[:], g_ps[:])
                return gt_sb

            def emit_z_part(xa, q, part, zp):
                # one quarter of the k2 range (4 DoubleRow matmuls)
                for k2 in range(NP // 4 * part, NP // 4 * (part + 1)):
                    nc.tensor.matmul(
                        zp[:],
                        az[q][:, k2 * 256:(k2 + 1) * 256]
                        .rearrange("p (two m) -> p two m", two=2),
                        xa[:, 2 * k2 * T:(2 * k2 + 2) * T]
                        .rearrange("p (two t) -> p two t", two=2),
                        start=(k2 == 0), stop=(k2 == NP - 1),
                        perf_mode=DR,
                    )

            def emit_gx_mul(st, q):
                gx = pp.tile([128, T], f32, space="PSUM", name="gx", tag="gx",
                             bufs=1)
                nc.tensor.matmul(gx[:],
                                 esb[:, q * 128:(q + 1) * 128],
                                 st["gt"][:],
                                 start=True, stop=True)
                nc.vector.tensor_mul(st["zbs"][q][:], st["zbs"][q][:], gx[:])

            def emit_delta_pairs(st, q, prs, veceng):
                # delta pair matmuls + PSUM->SBUF fp8 drains for pair
                # indices prs of quarter q of the previous macrotile
                for pr in prs:
                    hc = q * 8 + pr * 2
                    dp = pp.tile([128, 2 * T], f32, space="PSUM", name="dp",
                                 tag="dp", bufs=2)
                    for hf in range(2):
                        nc.tensor.matmul(
                            dp[:, hf * T:(hf + 1) * T],
                            bpk[:, (hc + hf) * 128:(hc + hf + 1) * 128],
                            st["zbs"][q][:],
                            start=True, stop=True)
                    if veceng == "both":
                        # tail drain: no later z-phase to hide behind, so
                        # both engines drain one half of every pair
                        nc.scalar.copy(st["da"][:, hc * T:(hc + 1) * T],
                                       dp[:, 0:T])
                        nc.vector.tensor_copy(
                            st["da"][:, (hc + 1) * T:(hc + 2) * T],
                            dp[:, T:2 * T])
                    elif pr in veceng:
                        nc.vector.tensor_copy(st["da"][:, hc * T:(hc + 2) * T],
                                              dp[:])
                    else:
                        nc.scalar.copy(st["da"][:, hc * T:(hc + 2) * T],
                                       dp[:])

            def emit_store(st, q):
                smt = st["mt"]
                nc.sync.dma_start(
                    yt_ap[:, (smt * 4 + q) * 8 * T:(smt * 4 + q + 1) * 8 * T],
                    st["da"][:, q * 8 * T:(q + 1) * 8 * T],
                )

            def emit_chain(stn_all, only_ts=None):
                # the DVE gating chain for token sub-tiles
                ggs = []
                tslist = range(NTS) if only_ts is None else [only_ts]
                for ts in tslist:
                    stn = stn_all[:, ts * 128:(ts + 1) * 128]
                    r1 = spool.tile([128, 1], f32, name="r1", tag="r1")
                    nc.vector.reduce_max(r1[:], stn, axis=AX.X)
                    s2 = gpool.tile([128, 128], f32, name="s2", tag="s2")
                    nc.vector.scalar_tensor_tensor(s2[:], stn, r1[:], stn,
                                                   AL.is_lt, AL.mult)
                    r2 = spool.tile([128, 1], f32, name="r2", tag="r2")
                    nc.vector.reduce_max(r2[:], s2[:], axis=AX.X)
                    s3 = gpool.tile([128, 128], f32, name="s3", tag="s3")
                    nc.vector.scalar_tensor_tensor(s3[:], s2[:], r2[:], s2[:],
                                                   AL.is_lt, AL.mult)
                    r3 = spool.tile([128, 1], f32, name="r3", tag="r3")
                    nc.vector.reduce_max(r3[:], s3[:], axis=AX.X)
                    nr1 = spool.tile([128, 1], f32, name="nr1", tag="nr1")
                    nc.vector.tensor_scalar_mul(nr1[:], r1[:], -1.0)
                    ex = gpool.tile([128, 128], f32, name="ex", tag="ex")
                    nc.scalar.activation(ex[:], stn, AF.Exp, bias=nr1[:],
                                         scale=1.0)
                    em = gpool.tile([128, 128], f32, name="em", tag="em")
                    zs = spool.tile([128, 1], f32, name="zs", tag="zs")
                    nc.vector.scalar_tensor_tensor(em[:], stn, r3[:], ex[:],
                                                   AL.is_ge, AL.mult,
                                                   accum_out=zs[:])
                    rz = spool.tile([128, 1], f32, name="rz", tag="rz")
                    nc.vector.reciprocal(rz[:], zs[:])
                    gg = gpool.tile([128, 128], bf16, name="gg", tag="gg",
                                    bufs=NTS + 1)
                    nc.vector.tensor_scalar_mul(gg[:], em[:], rz[:])
                    ggs.append(gg)
                return ggs

            def emit_chain_ts(stn_all, ts):
                return emit_chain(stn_all, only_ts=ts)[0]

            # ---------------- the pipelined macrotile loop ----------------
            prev = None  # state of the previous macrotile
            lastggs = []
            for mt in range(NMT):
                xa = xpool.tile([128, NKT * T], f8, name="xa", tag="xa")
                PART = NKT * T // 4
                for part in range(4):
                    nc.sync.dma_start(
                        xa[:, part * PART:(part + 1) * PART],
                        xt_ap[:, mt * NKT * T + part * PART:
                              mt * NKT * T + (part + 1) * PART])
                if mt == 0:
                    emit_const_loads()
                    # the PE idles ~5us waiting for xa0; zero-matmuls ramp
                    # the HAM clock gate so mt0 runs at full rate
                    wp = pp.tile([128, T], f32, space="PSUM", name="gx",
                                 tag="gx", bufs=1)
                    for _ in range(10):
                        nc.tensor.matmul(wp[:], ident[:], warmt[:],
                                         start=True, stop=True)

                s_sb = emit_scores(xa)
                stn_all = emit_transpose_scores(s_sb)
                if prev is not None:
                    prev["gt"] = emit_transpose_gates(prev["ggs"])

                da = dpool.tile([128, NKT * T], f8, name="da", tag="da")
                cur = {"mt": mt, "da": da, "zbs": [], "ggs": None}

                for q in range(4):
                    vq = {1, 3} if q % 2 else {1}
                    if prev is not None:
                        emit_gx_mul(prev, q)
                        emit_delta_pairs(prev, q, (0,), veceng=vq)
                    zp = pp.tile([128, T], f32, space="PSUM", name="zp",
                                 tag="zp")
                    for part in range(4):
                        emit_z_part(xa, q, part, zp)
                        if prev is not None and part < 3:
                            emit_delta_pairs(prev, q, (part + 1,), veceng=vq)
                    if prev is not None:
                        emit_store(prev, q)
                    zb = zpool.tile([128, T], bf16, name="zb", tag="zb")
                    nc.scalar.activation(zb[:], zp[:], AF.Identity,
                                         bias=bias[:, q:q + 1], scale=1.0)
                    cur["zbs"].append(zb)
                    if mt == NMT - 1:
                        # last macrotile: its chain competes with the tail,
                        # so spread the sub-chains across the z quarters
                        lastggs.append(emit_chain_ts(stn_all, q))

                if mt == NMT - 1:
                    cur["ggs"] = lastggs
                else:
                    cur["ggs"] = emit_chain(stn_all)
                prev = cur

            # ---- drain the pipeline: delta of the last macrotile ----
            prev["gt"] = emit_transpose_gates(prev["ggs"])
            # tail drain: all 4 gate-expand matmuls + zg muls run first
            # (each mul frees the single gx bank; the vector queue is empty
            # so the round-trips are short), then the delta matmul stream
            # runs without any vector-queue interruptions.  The z-phase
            # "zp" PSUM tiles are dead now, so the delta chunks cycle
            # through dp pairs AND zp singles (6 chunks in flight).
            for q in range(4):
                emit_gx_mul(prev, q)
            for q in range(4):
                zgq = prev["zbs"][q]
                for grp in range(2):           # chunks [0..3] then [4..7]
                    hc0 = q * 8 + grp * 4
                    dp = pp.tile([128, 2 * T], f32, space="PSUM", name="dp",
                                 tag="dp", bufs=2)
                    for hf in range(2):
                        nc.tensor.matmul(
                            dp[:, hf * T:(hf + 1) * T],
                            bpk[:, (hc0 + hf) * 128:(hc0 + hf + 1) * 128],
                            zgq[:], start=True, stop=True)
                    zsingles = []
                    for sng in range(2):
                        hc = hc0 + 2 + sng
                        zps = pp.tile([128, T], f32, space="PSUM", name="zp",
                                      tag="zp")
                        nc.tensor.matmul(zps[:],
                                         bpk[:, hc * 128:(hc + 1) * 128],
                                         zgq[:], start=True, stop=True)
                        zsingles.append((hc, zps))
                    # drain the zp singles FIRST (the next group's single
                    # matmuls wait on this 2-buf rotation), pairs after
                    for sng, (hc, zps) in enumerate(zsingles):
                        dsl = prev["da"][:, hc * T:(hc + 1) * T]
                        if sng == 0:
                            nc.scalar.copy(dsl, zps[:])
                        else:
                            nc.vector.tensor_copy(dsl, zps[:])
                    nc.scalar.copy(prev["da"][:, hc0 * T:(hc0 + 1) * T],
                                   dp[:, 0:T])
                    nc.vector.tensor_copy(
                        prev["da"][:, (hc0 + 1) * T:(hc0 + 2) * T],
                        dp[:, T:2 * T])
                    # store each 4-chunk group as soon as it drains so the
                    # final store (and its ~2us completion latency) is small
                    smt = prev["mt"]
                    off = (smt * 4 + q) * 8 * T + grp * 4 * T
                    nc.sync.dma_start(
                        yt_ap[:, off:off + 4 * T],
                        prev["da"][:, hc0 * T:(hc0 + 4) * T],
                    )

    nc.compile()
    return nc


def _prep_consts(task_emb, task_ids, Wp, bp, centers, A, Bm, adapter_scale):
    scale = float(np.asarray(adapter_scale))
    A_all = np.ascontiguousarray(
        A.transpose(1, 0, 2).reshape(H, NB * R).astype(np.float32))
    W2 = (Wp @ centers.T).astype(np.float32)                     # [H, 128]

    # ah: [p, q, k2, two, m] = A_all[(2*k2+two)*128+p, q*128+m], fp8 e4m3
    # (DoubleRow pairs of consecutive k-tiles interleave along the free dim)
    ah = (A_all.reshape(NKT, 128, 4, 128).transpose(1, 2, 0, 3)
          .reshape(128, 4 * NKT * 128).astype(ml_dtypes.float8_e4m3))
    ah = np.ascontiguousarray(ah)
    # ws: [p, k2, two, m] = W2[(2*k2+two)*128+p, m], fp8 (DoubleRow pairs)
    wsn = np.ascontiguousarray(
        W2.reshape(NKT, 128, 128).transpose(1, 0, 2).reshape(128, NKT * 128)
        .astype(ml_dtypes.float8_e4m3))

    # block-diag up-projection, K=128 per h-chunk
    bpk = np.zeros((128, NKT * 128), np.float32)
    for hc in range(NKT):
        for mblk in range(4):
            n = hc * 4 + mblk
            for r in range(R):
                row = (hc % 8) * 16 + mblk * 4 + r
                bpk[row, hc * 128 + mblk * 32: hc * 128 + mblk * 32 + 32] = \
                    Bm[n, r, :] * scale
    bpk = bpk.astype(ml_dtypes.bfloat16)

    e_np = (np.arange(128)[:, None] == (np.arange(512)[None, :] // 4)) \
        .astype(ml_dtypes.bfloat16)

    sconst = (bp @ centers.T - 0.5 * (centers ** 2).sum(-1)).astype(np.float32)

    biases = []
    for c in range(NCORES):
        te = task_emb[int(np.asarray(task_ids)[c // 2])].astype(np.float32)
        b5 = np.empty((128, 5), np.float32)
        zoff = te @ A_all                                        # [512]
        for q in range(4):
            b5[:, q] = zoff[q * 128:(q + 1) * 128]
        b5[:, 4] = te @ W2 + sconst
        biases.append(np.ascontiguousarray(b5))
    return ah, wsn, bpk, e_np, biases


def kernel(x, task_ids, task_emb, Wp, bp, centers, A, Bm, adapter_scale):
    global _COMPILED, LAST_RESULT
    from concourse import bass_utils

    x = np.asarray(x, dtype=np.float32)
    task_ids = np.asarray(task_ids)
    task_emb = np.asarray(task_emb, dtype=np.float32)
    Wp = np.asarray(Wp, dtype=np.float32)
    bp = np.asarray(bp, dtype=np.float32)
    centers = np.asarray(centers, dtype=np.float32)
    A = np.asarray(A, dtype=np.float32)
    Bm = np.asarray(Bm, dtype=np.float32)

    if _COMPILED is None:
        _COMPILED = _build()
    nc = _COMPILED

    ah, wsn, bpk, e_np, biases = _prep_consts(
        task_emb, task_ids, Wp, bp, centers, A, Bm, adapter_scale)

    xf = x.reshape(B * S, H)
    xf8 = xf.astype(ml_dtypes.float8_e4m3)
    in_maps = []
    for c in range(NCORES):
        xtc = xf8[c * TPC:(c + 1) * TPC].reshape(NMT, T, NKT, 128)
        xtc = np.ascontiguousarray(xtc.transpose(3, 0, 2, 1)) \
            .reshape(128, NMT * NKT * T)
        in_maps.append({"xt": xtc, "ah": ah, "ws": wsn, "bpk": bpk,
                       "e": e_np, "bias": biases[c]})

    kwargs = {}
    if TRACE:
        kwargs = dict(trace=True, tmpdir=TRACE_DIR)
    res = bass_utils.run_bass_kernel_spmd(
        nc, in_maps, core_ids=list(range(NCORES)), **kwargs)
    LAST_RESULT = res

    out = np.empty((B * S, H), np.float32)
    for c in range(NCORES):
        dat = res.results[c]["yt"].reshape(128, NMT, 4, 8, T)
        delta = dat.transpose(1, 4, 2, 3, 0).reshape(TPC, H)
        out[c * TPC:(c + 1) * TPC] = xf[c * TPC:(c + 1) * TPC] + \
            delta.astype(np.float32)
    return out.reshape(B, S, H)
